# revision 15
# baseline (speedup 1.0000x reference)
"""BiAttentionMRU Trainium2 kernel.

Data-parallel over batch: B=16 -> 2 batch elements on each of 8 cores.
All weights replicated. Embedding gather done on-device via indirect DMA.

Layouts (per core, per batch element b in {0,1}):
  art gathered as [t-chunk(128), d=300], PE-transposed into artT[100, 3, 2000]
  (d on partitions, 3 chunks of 100). Group sums, z/o/CE matmuls, gate mix
  (all on PE as scaled-identity accumulating matmuls), MRU scan (native
  tensor_tensor_scan along t) and the attention lhsT all work in [d, t].

Attention algebra: aoq is never materialized. With e1 = exp(art_enc @ keys1^T),
Z1 its row sum, s2 = softmax-normalized p1 @ (q @ keys_f^T) is computed as
exp-of(u2 * 1/Z1) where u2 = e1 @ QK. The per-option mean over t of
softmax(s2) @ opt folds into one accumulating matmul sum_t e2[t,:] * (1/Z2[t]),
accumulated per (fi,o) column directly in PSUM (no partition-shift DMAs).

f1/f2/f3 biases are constant over their softmax axis (shift invariance)
and are dropped.

Per-b flow: phase A stashes the four group-sum tensors for all 3 d-chunks;
phase B runs a dc-local pipeline zob -> CE -> gate mix -> MRU scan so all
big transients recycle through bufs=2 pool tags.
"""

import sys

sys.path.insert(0, "/opt/trn_rl_repo")

import numpy as np
import ml_dtypes

import concourse.bass as bass
import concourse.tile as tile
from concourse import bacc, mybir
from concourse.masks import make_identity

F32 = mybir.dt.float32
BF16 = mybir.dt.bfloat16
I32 = mybir.dt.int32
AX = mybir.AxisListType
OP = mybir.AluOpType
AF = mybir.ActivationFunctionType

DIM = 300
VOCAB = 50000
B_FULL = 16
NCORES = 8
BPC = B_FULL // NCORES  # batch per core = 2
T = 2000
TQ = 30
TO = 16
RANGES = (1, 2, 4, 10, 25)

TCH = [128] * 15 + [80]  # t chunking for transposes / attention
NTCH = len(TCH)
DC = 3  # d chunks of 100
DCS = 100

N_MM = 500  # matmul N-chunking for [d,t] streams (psum free <= 512 fp32)

USE_BF16 = True
DT = BF16 if USE_BF16 else F32
NPDT = ml_dtypes.bfloat16 if USE_BF16 else np.float32

# packed weight columns (bf16, [DIM, WCOLS] host-packed)
WC_ART = 0        # 900: Wz.T | Wo.T | ce0.T
WC_CE = 900       # 1200: ce1..ce4 transposed
WC_F1 = 2100      # 300: f1_W.T
WC_F2 = 2400      # 300: f2_W (untransposed)
WC_F3 = 2700      # 300: f3_W (untransposed)
WCOLS = 3000

# packed f32 bias columns ([DIM, FCOLS]): 0 bz, 1 bo, 2..6 ce_b[0..4]
FCOLS = 7

# scalar table columns (host-packed, replicated down 128 partitions)
SC_M1 = 0      # 15 cols: m1[k,r]/r at 5k+ri
SC_M1B = 15    # 3 cols
SC_M2 = 18     # 3 cols
SC_M2B = 21    # 1 col
SC_AS2B = 22   # 1 col
SC_NCOL = 24


DEBUG = False  # adds intermediate DRAM dumps (b=0) for numerics bisection
_DBG_SPECS = [
    ("d_artT", [DCS, 128]), ("d_xs2", [DCS, 64]), ("d_xs25", [DCS, 80]),
    ("d_z", [DCS, 128]), ("d_b1", [DCS, 128]), ("d_bl", [DCS, 1780]),
    ("d_h10", [DCS, 128]), ("d_h11", [DCS, 128]), ("d_h12", [DCS, 128]),
    ("d_gate", [DCS, 128]), ("d_ct", [DCS, 128]), ("d_ctend", [DCS, 128]),
    ("d_enc", [DCS, 128]), ("d_k1T", [DCS, DC * TQ]), ("d_qk", [TQ, 128]),
    ("d_pb", [TO, 8]), ("d_ans", [DCS, BPC * 24]),
    ("d_enc1", [DCS, 128]), ("d_enc2", [DCS, 128]),
    ("d_s1", [TQ, 128]), ("d_e1", [TQ, 128]), ("d_e2", [128, 128]),
    ("d_z2", [128, 16]),
]


def _build_program():
    nc = bacc.Bacc("TRN2", target_bir_lowering=False, debug=False,
                   num_devices=NCORES)

    emb = nc.dram_tensor("emb", [VOCAB, DIM], DT, kind="ExternalInput")
    art_idx = nc.dram_tensor("art_idx", [BPC, T], I32, kind="ExternalInput")
    q_idx = nc.dram_tensor("q_idx", [BPC, TQ], I32, kind="ExternalInput")
    opt_idx = nc.dram_tensor("opt_idx", [BPC, 4, TO], I32, kind="ExternalInput")
    wpack = nc.dram_tensor("wpack", [DIM, WCOLS], DT, kind="ExternalInput")
    fpack = nc.dram_tensor("fpack", [DIM, FCOLS], F32, kind="ExternalInput")
    as1p = nc.dram_tensor("as1p", [2 * DIM, 75], F32, kind="ExternalInput")
    asm = nc.dram_tensor("asm", [75, 2], F32, kind="ExternalInput")
    scal = nc.dram_tensor("scal", [128, SC_NCOL], F32, kind="ExternalInput")
    out = nc.dram_tensor("scores", [BPC, 4], F32, kind="ExternalOutput")
    dbg = {}
    if DEBUG:
        for nm, shp in _DBG_SPECS:
            dbg[nm] = nc.dram_tensor(nm, shp, F32, kind="ExternalOutput")

    with tile.TileContext(nc) as tc:
        from contextlib import ExitStack
        with ExitStack() as ctx:
            _emit(nc, tc, ctx, emb, art_idx, q_idx, opt_idx, wpack, fpack,
                  as1p, asm, scal, out, dbg)

    nc.compile()
    return nc


def _emit(nc, tc, ctx, emb, art_idx, q_idx, opt_idx, wpack, fpack, as1p,
          asm, scal, out, dbg=None):
    def dump(nm, ap):
        if dbg:
            t = pers.tile(dict(_DBG_SPECS)[nm], F32, tag=f"dmp{nm}")
            nc.vector.tensor_copy(t[:], ap)
            nc.sync.dma_start(dbg[nm][:], t[:])
    # ---------------- pools ----------------
    consts = ctx.enter_context(tc.tile_pool(name="consts", bufs=1))
    gpool = ctx.enter_context(tc.tile_pool(name="gather", bufs=3))
    p_art = ctx.enter_context(tc.tile_pool(name="p_art", bufs=1))
    p_enc = ctx.enter_context(tc.tile_pool(name="p_enc", bufs=1))
    p_tr = ctx.enter_context(tc.tile_pool(name="p_tr", bufs=2))
    pers = ctx.enter_context(tc.tile_pool(name="pers", bufs=1))
    small = ctx.enter_context(tc.tile_pool(name="small", bufs=4))
    # PSUM budget (8 banks): mm(3) + work(3) + pb(2). Every psum tag must
    # stay within this set -- each tag costs bufs banks.
    pp500 = ctx.enter_context(tc.tile_pool(name="pp500", bufs=3, space="PSUM"))
    ppwork = ctx.enter_context(tc.tile_pool(name="ppwork", bufs=3, space="PSUM"))
    ppacc = ctx.enter_context(tc.tile_pool(name="ppacc", bufs=2, space="PSUM"))

    # ---------------- index loads (sync queue, first) ----------------
    aidx = []
    for b in range(BPC):
        ai = small.tile([128, NTCH], I32, tag=f"aidx{b}", name=f"aidx{b}")
        nc.sync.dma_start(ai[:, 0:NTCH - 1],
                          art_idx[b, 0:(NTCH - 1) * 128]
                          .rearrange("(c p) -> p c", p=128))
        nc.sync.dma_start(ai[:TCH[-1], NTCH - 1:NTCH],
                          art_idx[b, (NTCH - 1) * 128:T, None])
        aidx.append(ai)
    qidx = small.tile([TQ, BPC], I32, tag="qidx")
    nc.sync.dma_start(qidx[:], q_idx[:].rearrange("b w -> w b"))
    oidx = small.tile([TO, BPC, 4], I32, tag="oidx")
    nc.sync.dma_start(oidx[:], opt_idx[:].rearrange("b o w -> w b o"))

    # ---------------- constants / weights (sync queue, after idx) ------
    ident = consts.tile([128, 128], DT)
    make_identity(nc, ident[:])

    w_sb = consts.tile([DCS, DC, WCOLS], DT)
    nc.sync.dma_start(w_sb[:], wpack[:].rearrange("(c p) x -> p c x", p=DCS))
    bias_sb = consts.tile([DCS, DC, FCOLS], F32)
    nc.sync.dma_start(bias_sb[:], fpack[:].rearrange("(c p) x -> p c x", p=DCS))
    scal_sb = consts.tile([128, SC_NCOL], F32)
    nc.sync.dma_start(scal_sb[:], scal[:])
    w_as1_sb = consts.tile([DCS, 6, 75], F32)
    nc.sync.dma_start(w_as1_sb[:], as1p[:].rearrange("(c p) x -> p c x", p=DCS))
    asm_sb = consts.tile([75, 2], F32)
    nc.sync.dma_start(asm_sb[:], asm[:])

    def sc(col):  # f32 per-partition scalar [100,1]
        return scal_sb[0:DCS, col:col + 1]

    ones30 = consts.tile([TQ, 1], DT)
    nc.vector.memset(ones30[:], 1.0)
    # scaled 100x100 identities for the PE-side gate mix:
    # cols j=5k+ri hold m1[k,ri]/r * I, cols 15+k hold m2[k] * I
    mI = consts.tile([DCS, 18, DCS], DT)
    for j in range(18):
        scol = (SC_M1 + j) if j < 15 else (SC_M2 + j - 15)
        nc.vector.tensor_scalar_mul(mI[:, j, :], ident[0:DCS, 0:DCS], sc(scol))

    ans_sb = pers.tile([DCS, BPC, 6, 4], F32, tag="ans_sb")

    # ---------------- gathers + transposes (both b first) --------------
    artT = []
    og = []
    qT = pers.tile([DCS, DC, BPC, TQ], DT, tag="qT")
    oT = pers.tile([DCS, DC, BPC, 4, TO], DT, tag="oT")
    for b in range(BPC):
        at = p_art.tile([DCS, DC, T], DT, tag=f"artT{b}", name=f"artT{b}")
        artT.append(at)
        for c in range(NTCH):
            pc = TCH[c]
            g = gpool.tile([128, DIM], DT, tag="gart", name="gart")
            nc.gpsimd.indirect_dma_start(
                out=g[:pc, :], out_offset=None, in_=emb[:],
                in_offset=bass.IndirectOffsetOnAxis(ap=aidx[b][:pc, c:c + 1],
                                                    axis=0))
            for dc in range(DC):
                tp = ppwork.tile([DCS, 128], DT, tag="work")
                nc.tensor.transpose(tp[:, :pc],
                                    g[:pc, dc * DCS:(dc + 1) * DCS],
                                    ident[:pc, :pc])
                nc.vector.tensor_copy(at[:, dc, c * 128:c * 128 + pc],
                                      tp[:, :pc])

        qgb = pers.tile([TQ, DIM], DT, tag=f"qg{b}", name=f"qg{b}")
        nc.gpsimd.indirect_dma_start(
            out=qgb[:], out_offset=None, in_=emb[:],
            in_offset=bass.IndirectOffsetOnAxis(ap=qidx[:, b:b + 1], axis=0))
        for dc in range(DC):
            tp = ppwork.tile([DCS, 128], DT, tag="work")
            nc.tensor.transpose(tp[:, :TQ], qgb[:, dc * DCS:(dc + 1) * DCS],
                                ident[:TQ, :TQ])
            nc.vector.tensor_copy(qT[:, dc, b, :], tp[:, :TQ])

        ogb = [pers.tile([TO, DIM], DT, tag=f"og{b}_{o}", name=f"og{b}_{o}")
               for o in range(4)]
        og.append(ogb)
        for o in range(4):
            nc.gpsimd.indirect_dma_start(
                out=ogb[o][:], out_offset=None, in_=emb[:],
                in_offset=bass.IndirectOffsetOnAxis(ap=oidx[:, b, o:o + 1],
                                                    axis=0))
            for dc in range(DC):
                tp = ppwork.tile([DCS, 128], DT, tag="work")
                nc.tensor.transpose(tp[:, :TO],
                                    ogb[o][:, dc * DCS:(dc + 1) * DCS],
                                    ident[:TO, :TO])
                nc.vector.tensor_copy(oT[:, dc, b, o, :], tp[:, :TO])

    def drain_relu(dst, ps, bias_ap, eng):
        """psum -> sbuf relu(x + bias). eng: 's' Scalar ACT, 'v' DVE."""
        if eng == "s":
            nc.scalar.activation(dst, ps, AF.Relu, bias=bias_ap)
        else:
            nc.vector.tensor_scalar(dst, ps, bias_ap, 0.0,
                                    op0=OP.add, op1=OP.max)

    # ---------------- main per-b stream ----------------
    encT = []
    GSZ = (T // 2, T // 4, T // 10, T // 25)
    for b in range(BPC):
        at = artT[b]
        o_full = p_enc.tile([DCS, DC, T], DT, tag=f"enc{b}", name=f"enc{b}")
        encT.append(o_full)

        # ---- phase A: group sums for all 3 d-chunks ----
        xs2 = p_tr.tile([DCS, DC, T // 2], DT, tag="xs2", name="xs2")
        xs4 = p_tr.tile([DCS, DC, T // 4], DT, tag="xs4", name="xs4")
        xs10 = p_tr.tile([DCS, DC, T // 10], DT, tag="xs10", name="xs10")
        xs25 = p_tr.tile([DCS, DC, T // 25], DT, tag="xs25", name="xs25")
        xs = (xs2, xs4, xs10, xs25)
        for dc in range(DC):
            a = at[:, dc, :]
            nc.gpsimd.tensor_add(xs2[:, dc, :], a[:, 0:T:2], a[:, 1:T:2])
            nc.gpsimd.tensor_add(xs4[:, dc, :], xs2[:, dc, 0:T // 2:2],
                                 xs2[:, dc, 1:T // 2:2])
            with nc.allow_low_precision(reason="bf16 group sums, bf16 mms"):
                nc.vector.tensor_reduce(
                    xs10[:, dc, :],
                    xs2[:, dc, :].rearrange("p (g r) -> p g r", r=5),
                    AX.X, OP.add)
                nc.vector.tensor_reduce(
                    xs25[:, dc, :],
                    a[:].rearrange("p (g r) -> p g r", r=25),
                    AX.X, OP.add)
            if b == 0 and dc == 0:
                dump("d_artT", a[:, 0:128])
                dump("d_xs2", xs2[:, 0, 0:64])
                dump("d_xs25", xs25[:, 0, :])

        # ---- phase B: dc-local pipeline ----
        for dc in range(DC):
            # z / o / B1 for this output-dc
            z_sb = p_tr.tile([DCS, T], DT, tag="z", name="z_sb")
            b1_sb = p_tr.tile([DCS, T], DT, tag="b1", name="b1_sb")
            for mi, (dst, func, bcol) in enumerate(
                    ((z_sb[:], AF.Tanh, 0), (o_full[:, dc, :], AF.Tanh, 1),
                     (b1_sb[:], AF.Relu, 2))):
                mcol = WC_ART + mi * DIM + dc * DCS
                for t0 in range(0, T, N_MM):
                    ps = pp500.tile([DCS, N_MM], F32, tag="mm")
                    for kc in range(DC):
                        nc.tensor.matmul(
                            ps[:], w_sb[:, kc, mcol:mcol + DCS],
                            at[:, kc, t0:t0 + N_MM],
                            start=(kc == 0), stop=(kc == DC - 1))
                    if mi == 2:
                        drain_relu(dst[:, t0:t0 + N_MM], ps[:],
                                   bias_sb[:, dc, bcol:bcol + 1], "v")
                    else:
                        nc.scalar.activation(
                            dst[:, t0:t0 + N_MM], ps[:], func,
                            bias=bias_sb[:, dc, bcol:bcol + 1])

            # CE r>=2 for this output-dc
            bl = p_tr.tile([DCS, sum(GSZ)], DT, tag="bl", name="bl")
            boff = [0, T // 2, T // 2 + T // 4, T // 2 + T // 4 + T // 10]
            for ri in range(4):
                g_r = GSZ[ri]
                wcol = WC_CE + ri * DIM + dc * DCS
                for j, g0 in enumerate(range(0, g_r, N_MM)):
                    gn = min(N_MM, g_r - g0)
                    ps = pp500.tile([DCS, N_MM], F32, tag="mm")
                    for kc in range(DC):
                        nc.tensor.matmul(
                            ps[:, :gn], w_sb[:, kc, wcol:wcol + DCS],
                            xs[ri][:, kc, g0:g0 + gn],
                            start=(kc == 0), stop=(kc == DC - 1))
                    drain_relu(bl[:, boff[ri] + g0:boff[ri] + g0 + gn],
                               ps[:, :gn], bias_sb[:, dc, 3 + ri:4 + ri],
                               "s" if (ri + j) % 2 == 0 else "v")

            if b == 0 and dc == 0:
                dump("d_z", z_sb[:, 0:128])
                dump("d_b1", b1_sb[:, 0:128])
                dump("d_bl", bl[:])

            def ev_chunk(ri, t0, tn):
                r = RANGES[ri]
                if r == 1:
                    return b1_sb[:, t0:t0 + tn]
                return bl[:, boff[ri - 1] + t0 // r:
                          boff[ri - 1] + (t0 + tn) // r, None] \
                    .to_broadcast([DCS, tn // r, r])

            # gate mix: h1_k = relu(sum_r m1[k,r]/r * B_r^expand + m1_b[k]);
            # gate = relu(sum_k m2[k] h1_k + m2_b). All on PE via scaled
            # identities; expansion = stride-0 rhs views.
            h1 = []
            for k in range(3):
                acc = p_tr.tile([DCS, T], DT, tag=f"h1_{k}", name=f"h1_{k}")
                for t0 in range(0, T, N_MM):
                    ps = pp500.tile([DCS, N_MM], F32, tag="mm")
                    for ri in range(5):
                        nc.tensor.matmul(ps[:], mI[:, 5 * k + ri, :],
                                         ev_chunk(ri, t0, N_MM),
                                         start=(ri == 0), stop=(ri == 4))
                    drain_relu(acc[:, t0:t0 + N_MM], ps[:], sc(SC_M1B + k),
                               "s" if k == 0 or (k == 2 and t0 < T // 2)
                               else "v")
                h1.append(acc)
            if b == 0 and dc == 0:
                dump("d_h10", h1[0][:, 0:128])
                dump("d_h11", h1[1][:, 0:128])
                dump("d_h12", h1[2][:, 0:128])
            gate = p_tr.tile([DCS, T], DT, tag="gate", name="gate")
            for t0 in range(0, T, N_MM):
                ps = pp500.tile([DCS, N_MM], F32, tag="mm")
                for k in range(3):
                    nc.tensor.matmul(ps[:], mI[:, 15 + k, :],
                                     h1[k][:, t0:t0 + N_MM],
                                     start=(k == 0), stop=(k == 2))
                drain_relu(gate[:, t0:t0 + N_MM], ps[:], sc(SC_M2B), "v")

            # MRU scan + encode for this dc
            if b == 0 and dc == 0:
                dump("d_gate", gate[:, 0:128])
            gz = p_tr.tile([DCS, T], DT, tag="gz", name="gz")
            nc.gpsimd.tensor_tensor(gz[:], gate[:], z_sb[:], op=OP.mult)
            nc.vector.tensor_sub(z_sb[:], z_sb[:], gz[:])  # (1-g)z
            c_t = p_tr.tile([DCS, T], DT, tag="c", name="c_t")
            nc.vector.tensor_tensor_scan(
                c_t[:], gate[:], z_sb[:], 0.0, op0=OP.mult, op1=OP.add)
            if b == 0 and dc == 0:
                dump("d_ct", c_t[:, 0:128])
                dump("d_ctend", c_t[:, T - 128:T])
            nc.vector.tensor_mul(o_full[:, dc, :], o_full[:, dc, :], c_t[:])
            if b == 0 and dc == 0:
                dump("d_enc", o_full[:, 0, 0:128])
            if b == 0 and dc == 1:
                dump("d_enc1", o_full[:, 1, 0:128])
            if b == 0 and dc == 2:
                dump("d_enc2", o_full[:, 2, 0:128])

    # ---------------- attention ----------------
    k1T, qk_sb = [], []
    for b in range(BPC):
        # keys1T = f1W @ q^T  (3 output-dc chunks)
        kT = pers.tile([DCS, DC, TQ], DT, tag=f"k1T{b}")
        k1T.append(kT)
        for dc in range(DC):
            ps = ppwork.tile([DCS, TQ], F32, tag="work")
            for kc in range(DC):
                nc.tensor.matmul(ps[:],
                                 w_sb[:, kc, WC_F1 + dc * DCS:
                                      WC_F1 + (dc + 1) * DCS],
                                 qT[:, kc, b, :],
                                 start=(kc == 0), stop=(kc == DC - 1))
            nc.scalar.copy(kT[:, dc, :], ps[:])

        # A2/A3 = q @ fW  then transpose; QK[q, (fi,o,w)] accumulation
        aTs = []
        for fi, wc in enumerate((WC_F2, WC_F3)):
            a_ps = ppwork.tile([TQ, DIM], F32, tag="work")
            for kc in range(DC):
                nc.tensor.matmul(a_ps[:], qT[:, kc, b, :],
                                 w_sb[:, kc, wc:wc + DIM],
                                 start=(kc == 0), stop=(kc == DC - 1))
            a_sb = small.tile([TQ, DIM], DT, tag="a_sb")
            nc.vector.tensor_copy(a_sb[:], a_ps[:])
            aT = small.tile([DCS, DC, TQ], DT, tag=f"aT{fi}")
            for dc in range(DC):
                tp = ppwork.tile([DCS, 128], DT, tag="work")
                nc.tensor.transpose(tp[:, :TQ],
                                    a_sb[:, dc * DCS:(dc + 1) * DCS],
                                    ident[:TQ, :TQ])
                nc.vector.tensor_copy(aT[:, dc, :], tp[:, :TQ])
            aTs.append(aT)

        qk_ps = ppwork.tile([TQ, 128], F32, tag="work")
        for fi in range(2):
            for o in range(4):
                gcol = 16 * (4 * fi + o)
                for kc in range(DC):
                    nc.tensor.matmul(qk_ps[:, gcol:gcol + 16],
                                     aTs[fi][:, kc, :], oT[:, kc, b, o, :],
                                     start=(kc == 0), stop=(kc == DC - 1))
        qk = pers.tile([TQ, 128], DT, tag=f"qk_sb{b}")
        qk_sb.append(qk)
        nc.vector.tensor_copy(qk[:], qk_ps[:])
        if b == 0:
            dump("d_k1T", kT[:].rearrange("p a b -> p (a b)"))
            dump("d_qk", qk[:])

    # attention stream over t chunks; pb[(fi,o)] columns accumulate in PSUM.
    # start=True clears has_written for the WHOLE bank, so interleaved
    # column-groups must instead memset once and accumulate with
    # start=False throughout (accumulate-onto-zero == overwrite).
    pb_ps = [ppacc.tile([TO, 8], F32, tag="pb", name=f"pb{b}")
             for b in range(BPC)]
    for b in range(BPC):
        nc.vector.memset(pb_ps[b][:], 0.0)
    for c in range(NTCH):
        pc = TCH[c]
        for b in range(BPC):
            s1 = pp500.tile([TQ, 128], F32, tag="mm")
            for dc in range(DC):
                nc.tensor.matmul(s1[:, :pc], k1T[b][:, dc, :],
                                 encT[b][:, dc, c * 128:c * 128 + pc],
                                 start=(dc == 0), stop=(dc == DC - 1))
            if b == 0 and c == 0 and dbg:
                dump("d_s1", s1[:])
            e1T = small.tile([TQ, 128], DT, tag="e1Ts")
            nc.scalar.activation(e1T[:, :pc], s1[:, :pc], AF.Exp)
            if b == 0 and c == 0 and dbg:
                dump("d_e1", e1T[:])
            z1ps = ppwork.tile([128, 2], F32, tag="work")
            nc.tensor.matmul(z1ps[:pc, 0:1], e1T[:, :pc], ones30[:],
                             start=True, stop=True)
            z1 = small.tile([128, 2], F32, tag="z1")
            nc.vector.reciprocal(z1[:pc, 1:2], z1ps[:pc, 0:1])
            u2 = pp500.tile([128, 128], F32, tag="mm")
            nc.tensor.matmul(u2[:pc, :], e1T[:, :pc], qk_sb[b][:],
                             start=True, stop=True)
            e2 = small.tile([128, 128], F32, tag="e2")
            nc.scalar.activation(e2[:pc, :], u2[:pc, :], AF.Exp,
                                 scale=z1[:pc, 1:2])
            z2 = small.tile([128, 16], F32, tag="z2")
            nc.vector.tensor_reduce(
                z2[:pc, 0:8],
                e2[:pc, :].rearrange("p (g w) -> p g w", w=16),
                AX.X, OP.add)
            nc.vector.reciprocal(z2[:pc, 8:16], z2[:pc, 0:8])
            if b == 0 and c == 0 and dbg:
                dump("d_e2", e2[:])
                dump("d_z2", z2[:])
            for g in range(8):
                nc.tensor.matmul(pb_ps[b][:, g:g + 1],
                                 e2[:pc, 16 * g:16 * g + 16],
                                 z2[:pc, 8 + g:9 + g],
                                 start=False, stop=(c == NTCH - 1),
                                 skip_group_check=True)

    # ---------------- answer vectors ----------------
    for b in range(BPC):
        pb_sb = small.tile([TO, 8], DT, tag="pb_sb")
        nc.vector.tensor_copy(pb_sb[:], pb_ps[b][:])
        if b == 0 and dbg:
            dump("d_pb", pb_ps[b][:])
        ans_ps = ppacc.tile([DCS, 24], F32, tag="pb", name=f"ans{b}")
        for g in range(8):
            fi, o = g // 4, g % 4
            for dc in range(DC):
                j = fi * 3 + dc
                nc.tensor.matmul(ans_ps[:, j * 4 + o:j * 4 + o + 1],
                                 og[b][o][:, dc * DCS:(dc + 1) * DCS],
                                 pb_sb[:, g:g + 1],
                                 start=True, stop=True)
        # 1/T of the mean-over-t lands here
        nc.vector.tensor_scalar_mul(
            ans_sb[:, b, :, :].rearrange("p j o -> p (j o)"), ans_ps[:],
            1.0 / T)

    if dbg:
        dump("d_ans", ans_sb[:].rearrange("p a b c -> p (a b c)"))
    # ---------------- final MLP (both batches together) ----------------
    h_ps = ppwork.tile([75, 8], F32, tag="work")
    for j in range(6):
        # rhs columns = (b, o) pairs for chunk j of the 600-dim ans vector
        nc.tensor.matmul(h_ps[:], w_as1_sb[:, j, :], ans_sb[:, :, j, :],
                         start=(j == 0), stop=(j == 5))
    h_sb = small.tile([75, 8], F32, tag="h_sb")
    nc.scalar.activation(h_sb[:], h_ps[:], AF.Relu, bias=asm_sb[:, 0:1])
    s_ps = ppwork.tile([8, 1], F32, tag="work")
    nc.tensor.matmul(s_ps[:], h_sb[:], asm_sb[:, 1:2], start=True, stop=True)
    s_sb = small.tile([8, 1], F32, tag="s_sb")
    nc.scalar.activation(s_sb[:], s_ps[:], AF.Identity,
                         bias=scal_sb[0:8, SC_AS2B:SC_AS2B + 1])
    nc.sync.dma_start(out[:].rearrange("b o -> (b o)")[:, None], s_sb[:])


# ---------------------------------------------------------------------------
# host side
# ---------------------------------------------------------------------------

_CACHE = {}


def _get_nc():
    if "nc" not in _CACHE:
        _CACHE["nc"] = _build_program()
    return _CACHE["nc"]


def _prep_core_inputs(inputs, core):
    b0 = core * BPC
    sl = slice(b0, b0 + BPC)
    f = np.asarray
    prep = _CACHE.get("prep_shared")
    if prep is None:
        # core-independent tensors, computed once per kernel() call set
        Wz, Wo = f(inputs["Wz"]), f(inputs["Wo"])
        ceW = f(inputs["ce_W"])
        wpack = np.concatenate(
            [Wz.T, Wo.T, ceW[0].T] + [ceW[i].T for i in (1, 2, 3, 4)]
            + [f(inputs["f1_W"]).T, f(inputs["f2_W"]), f(inputs["f3_W"])],
            axis=1)
        fpack = np.stack(
            [f(inputs["bz"]), f(inputs["bo"]),
             *[f(inputs["ce_b"])[i] for i in range(5)]], axis=1)
        prep = {
            "emb": f(inputs["emb"]).astype(NPDT),
            "wpack": np.ascontiguousarray(wpack).astype(NPDT),
            "fpack": np.ascontiguousarray(fpack).astype(np.float32),
            "as1p": np.ascontiguousarray(f(inputs["as1_W"]).T).astype(
                np.float32),
            "asm": np.stack([f(inputs["as1_b"]),
                             f(inputs["as2_W"])[0]], axis=1).astype(
                np.float32),
        }
        scal = np.zeros((128, SC_NCOL), np.float32)
        m1 = f(inputs["mr1_W"])
        for k in range(3):
            for ri, r in enumerate(RANGES):
                scal[:, SC_M1 + 5 * k + ri] = m1[k, ri] / r
        scal[:, SC_M1B:SC_M1B + 3] = f(inputs["mr1_b"])[None, :]
        scal[:, SC_M2:SC_M2 + 3] = f(inputs["mr2_W"])[0][None, :]
        scal[:, SC_M2B] = f(inputs["mr2_b"])[0]
        scal[:, SC_AS2B] = f(inputs["as2_b"])[0]
        prep["scal"] = scal
        _CACHE["prep_shared"] = prep

    d = dict(prep)
    d["art_idx"] = f(inputs["article_in"])[sl].astype(np.int32)
    d["q_idx"] = f(inputs["question_in"])[sl].astype(np.int32)
    d["opt_idx"] = np.stack(
        [f(inputs[f"option{i}_in"])[sl] for i in (1, 2, 3, 4)],
        axis=1).astype(np.int32)
    return d


def _get_runner():
    """jit-compiled 8-core runner, built once per process."""
    if "runner" in _CACHE:
        return _CACHE["runner"]
    import jax
    from jax.sharding import Mesh, PartitionSpec
    from jax.experimental.shard_map import shard_map
    from concourse.bass2jax import (_bass_exec_p, install_neuronx_cc_hook,
                                    partition_id_tensor)

    install_neuronx_cc_hook()
    nc = _get_nc()
    pid_name = nc.partition_id_tensor.name if nc.partition_id_tensor else None

    in_names, out_names, out_avals, zero_outs = [], [], [], []
    for alloc in nc.m.functions[0].allocations:
        if not isinstance(alloc, mybir.MemoryLocationSet):
            continue
        name = alloc.memorylocations[0].name
        if alloc.kind == "ExternalInput":
            if name != pid_name:
                in_names.append(name)
        elif alloc.kind == "ExternalOutput":
            out_names.append(name)
            shape = tuple(alloc.tensor_shape)
            dtype = mybir.dt.np(alloc.dtype)
            out_avals.append(jax.core.ShapedArray(shape, dtype))
            zero_outs.append(np.zeros(shape, dtype))
    n_params = len(in_names)
    all_in_names = in_names + out_names
    if pid_name is not None:
        all_in_names = all_in_names + [pid_name]

    def _body(*args):
        operands = list(args)
        if pid_name is not None:
            operands.append(partition_id_tensor())
        outs = _bass_exec_p.bind(
            *operands, out_avals=tuple(out_avals), in_names=tuple(all_in_names),
            out_names=tuple(out_names), lowering_input_output_aliases=(),
            sim_require_finite=True, sim_require_nnan=True, nc=nc)
        return tuple(outs)

    devices = jax.devices()[:NCORES]
    mesh = Mesh(np.asarray(devices), ("core",))
    in_specs = (PartitionSpec("core"),) * (n_params + len(out_names))
    out_specs = (PartitionSpec("core"),) * len(out_names)
    sharded = jax.jit(shard_map(_body, mesh=mesh, in_specs=in_specs,
                                out_specs=out_specs, check_rep=False),
                      keep_unused=True)

    _CACHE["runner"] = (sharded, in_names, out_names, zero_outs)
    return _CACHE["runner"]


def run_cores(per_core_inputs):
    """per_core_inputs: list of 8 dicts name->np array. Returns out dicts."""
    sharded, in_names, out_names, zero_outs = _get_runner()
    concat_in = [np.concatenate([per_core_inputs[c][n] for c in range(NCORES)],
                                axis=0) for n in in_names]
    concat_zeros = [np.concatenate([z] * NCORES, axis=0) for z in zero_outs]
    outs = sharded(*concat_in, *concat_zeros)
    result = []
    for c in range(NCORES):
        d = {}
        for i, n in enumerate(out_names):
            arr = np.asarray(outs[i])
            per = arr.shape[0] // NCORES
            d[n] = arr[c * per:(c + 1) * per]
        result.append(d)
    return result


def prepare_device_args(per_core_inputs):
    """device_put the concatenated inputs once, for repeated timed runs."""
    import jax
    from jax.sharding import Mesh, PartitionSpec, NamedSharding
    sharded, in_names, out_names, zero_outs = _get_runner()
    devices = jax.devices()[:NCORES]
    mesh = Mesh(np.asarray(devices), ("core",))
    sh = NamedSharding(mesh, PartitionSpec("core"))
    concat_in = [np.concatenate([per_core_inputs[c][n] for c in range(NCORES)],
                                axis=0) for n in in_names]
    concat_zeros = [np.concatenate([z] * NCORES, axis=0) for z in zero_outs]
    args = [jax.device_put(a, sh) for a in concat_in + concat_zeros]
    jax.block_until_ready(args)
    return args


def run_prepared(dev_args):
    sharded, in_names, out_names, zero_outs = _get_runner()
    outs = sharded(*dev_args)
    import jax
    jax.block_until_ready(outs)
    return outs


def kernel(**inputs):
    _CACHE.pop("prep_shared", None)
    per_core = [_prep_core_inputs(inputs, c) for c in range(NCORES)]
    res = run_cores(per_core)
    out = np.concatenate([res[c]["scores"] for c in range(NCORES)], axis=0)
    return out.astype(np.float32)


# revision 18
# speedup vs baseline: 1.2773x; 1.2773x over previous
"""BiAttentionMRU Trainium2 kernel.

Data-parallel over batch: B=16 -> 2 batch elements on each of 8 cores.
All weights replicated. Embedding gather done on-device via indirect DMA.

Layouts (per core, per batch element b in {0,1}):
  art gathered as [t-chunk(128), d=300], PE-transposed into artT[100, 3, 2000]
  (d on partitions, 3 chunks of 100). Group sums, z/o/CE matmuls, gate mix
  (all on PE as scaled-identity accumulating matmuls), MRU scan (native
  tensor_tensor_scan along t) and the attention lhsT all work in [d, t].

Attention algebra: aoq is never materialized. With e1 = exp(art_enc @ keys1^T),
Z1 its row sum, s2 = softmax-normalized p1 @ (q @ keys_f^T) is computed as
exp-of(u2 * 1/Z1) where u2 = e1 @ QK. The per-option mean over t of
softmax(s2) @ opt folds into one accumulating matmul sum_t e2[t,:] * (1/Z2[t]),
accumulated per (fi,o) column directly in PSUM (no partition-shift DMAs).

f1/f2/f3 biases are constant over their softmax axis (shift invariance)
and are dropped.

Per-b flow: phase A stashes the four group-sum tensors for all 3 d-chunks;
phase B runs a dc-local pipeline zob -> CE -> gate mix -> MRU scan so all
big transients recycle through bufs=2 pool tags.
"""

import sys

sys.path.insert(0, "/opt/trn_rl_repo")

import numpy as np
import ml_dtypes

import concourse.bass as bass
import concourse.tile as tile
from concourse import bacc, mybir
from concourse.masks import make_identity

F32 = mybir.dt.float32
BF16 = mybir.dt.bfloat16
I32 = mybir.dt.int32
AX = mybir.AxisListType
OP = mybir.AluOpType
AF = mybir.ActivationFunctionType

DIM = 300
VOCAB = 50000
B_FULL = 16
NCORES = 8
BPC = B_FULL // NCORES  # batch per core = 2
T = 2000
TQ = 30
TO = 16
RANGES = (1, 2, 4, 10, 25)

TCH = [128] * 15 + [80]  # t chunking for transposes / attention
NTCH = len(TCH)
DC = 3  # d chunks of 100
DCS = 100

N_MM = 500  # matmul N-chunking for [d,t] streams (psum free <= 512 fp32)

USE_BF16 = True
DT = BF16 if USE_BF16 else F32
NPDT = ml_dtypes.bfloat16 if USE_BF16 else np.float32

# packed weight columns (bf16, [DIM, WCOLS] host-packed)
WC_ART = 0        # 900: Wz.T | Wo.T | ce0.T
WC_CE = 900       # 1200: ce1..ce4 transposed
WC_F1 = 2100      # 300: f1_W.T
WC_F2 = 2400      # 300: f2_W (untransposed)
WC_F3 = 2700      # 300: f3_W (untransposed)
WCOLS = 3000

# packed f32 bias columns ([DIM, FCOLS]): 0 bz, 1 bo, 2..6 ce_b[0..4]
FCOLS = 7

# scalar table columns (host-packed, replicated down 128 partitions)
SC_M1 = 0      # 15 cols: m1[k,r]/r at 5k+ri
SC_M1B = 15    # 3 cols
SC_M2 = 18     # 3 cols
SC_M2B = 21    # 1 col
SC_AS2B = 22   # 1 col
SC_NCOL = 24


DEBUG = False  # adds intermediate DRAM dumps (b=0) for numerics bisection
_DBG_SPECS = [
    ("d_artT", [DCS, 128]), ("d_xs2", [DCS, 64]), ("d_xs25", [DCS, 80]),
    ("d_z", [DCS, 128]), ("d_b1", [DCS, 128]), ("d_bl", [DCS, 1780]),
    ("d_h10", [DCS, 128]), ("d_h11", [DCS, 128]), ("d_h12", [DCS, 128]),
    ("d_gate", [DCS, 128]), ("d_ct", [DCS, 128]), ("d_ctend", [DCS, 128]),
    ("d_enc", [DCS, 128]), ("d_k1T", [DCS, DC * TQ]), ("d_qk", [TQ, 128]),
    ("d_pb", [TO, 8]), ("d_ans", [DCS, BPC * 24]),
    ("d_enc1", [DCS, 128]), ("d_enc2", [DCS, 128]),
    ("d_s1", [TQ, 128]), ("d_e1", [TQ, 128]), ("d_e2", [128, 128]),
    ("d_z2", [128, 16]),
]


def _build_program():
    nc = bacc.Bacc("TRN2", target_bir_lowering=False, debug=False,
                   num_devices=NCORES)

    emb = nc.dram_tensor("emb", [VOCAB, DIM], DT, kind="ExternalInput")
    art_idx = nc.dram_tensor("art_idx", [BPC, T], I32, kind="ExternalInput")
    q_idx = nc.dram_tensor("q_idx", [BPC, TQ], I32, kind="ExternalInput")
    opt_idx = nc.dram_tensor("opt_idx", [BPC, 4, TO], I32, kind="ExternalInput")
    wpack = nc.dram_tensor("wpack", [DIM, WCOLS], DT, kind="ExternalInput")
    fpack = nc.dram_tensor("fpack", [DIM, FCOLS], F32, kind="ExternalInput")
    as1p = nc.dram_tensor("as1p", [2 * DIM, 75], F32, kind="ExternalInput")
    asm = nc.dram_tensor("asm", [75, 2], F32, kind="ExternalInput")
    scal = nc.dram_tensor("scal", [128, SC_NCOL], F32, kind="ExternalInput")
    out = nc.dram_tensor("scores", [BPC, 4], F32, kind="ExternalOutput")
    dbg = {}
    if DEBUG:
        for nm, shp in _DBG_SPECS:
            dbg[nm] = nc.dram_tensor(nm, shp, F32, kind="ExternalOutput")

    with tile.TileContext(nc) as tc:
        from contextlib import ExitStack
        with ExitStack() as ctx:
            _emit(nc, tc, ctx, emb, art_idx, q_idx, opt_idx, wpack, fpack,
                  as1p, asm, scal, out, dbg)

    nc.compile()
    return nc


def _emit(nc, tc, ctx, emb, art_idx, q_idx, opt_idx, wpack, fpack, as1p,
          asm, scal, out, dbg=None):
    def dump(nm, ap):
        if dbg:
            t = pers.tile(dict(_DBG_SPECS)[nm], F32, tag=f"dmp{nm}")
            nc.vector.tensor_copy(t[:], ap)
            nc.sync.dma_start(dbg[nm][:], t[:])
    # ---------------- pools ----------------
    consts = ctx.enter_context(tc.tile_pool(name="consts", bufs=1))
    gpool = ctx.enter_context(tc.tile_pool(name="gather", bufs=3))
    p_art = ctx.enter_context(tc.tile_pool(name="p_art", bufs=1))
    p_enc = ctx.enter_context(tc.tile_pool(name="p_enc", bufs=1))
    p_tr = ctx.enter_context(tc.tile_pool(name="p_tr", bufs=2))
    pers = ctx.enter_context(tc.tile_pool(name="pers", bufs=1))
    small = ctx.enter_context(tc.tile_pool(name="small", bufs=4))
    # PSUM budget (8 banks): mm(3) + work(3) + pb(2). Every psum tag must
    # stay within this set -- each tag costs bufs banks.
    pp500 = ctx.enter_context(tc.tile_pool(name="pp500", bufs=3, space="PSUM"))
    ppwork = ctx.enter_context(tc.tile_pool(name="ppwork", bufs=3, space="PSUM"))
    ppacc = ctx.enter_context(tc.tile_pool(name="ppacc", bufs=2, space="PSUM"))

    # ---------------- index loads (sync queue, first) ----------------
    aidx = []
    for b in range(BPC):
        ai = small.tile([128, NTCH], I32, tag=f"aidx{b}", name=f"aidx{b}")
        nc.sync.dma_start(ai[:, 0:NTCH - 1],
                          art_idx[b, 0:(NTCH - 1) * 128]
                          .rearrange("(c p) -> p c", p=128))
        nc.sync.dma_start(ai[:TCH[-1], NTCH - 1:NTCH],
                          art_idx[b, (NTCH - 1) * 128:T, None])
        aidx.append(ai)
    qidx = small.tile([TQ, BPC], I32, tag="qidx")
    nc.sync.dma_start(qidx[:], q_idx[:].rearrange("b w -> w b"))
    oidx = small.tile([TO, BPC, 4], I32, tag="oidx")
    nc.sync.dma_start(oidx[:], opt_idx[:].rearrange("b o w -> w b o"))

    # ---------------- constants / weights (sync queue, after idx) ------
    ident = consts.tile([128, 128], DT)
    make_identity(nc, ident[:])

    w_sb = consts.tile([DCS, DC, WCOLS], DT)
    nc.sync.dma_start(w_sb[:], wpack[:].rearrange("(c p) x -> p c x", p=DCS))
    bias_sb = consts.tile([DCS, DC, FCOLS], F32)
    nc.sync.dma_start(bias_sb[:], fpack[:].rearrange("(c p) x -> p c x", p=DCS))
    scal_sb = consts.tile([128, SC_NCOL], F32)
    nc.sync.dma_start(scal_sb[:], scal[:])
    w_as1_sb = consts.tile([DCS, 6, 75], F32)
    nc.sync.dma_start(w_as1_sb[:], as1p[:].rearrange("(c p) x -> p c x", p=DCS))
    asm_sb = consts.tile([75, 2], F32)
    nc.sync.dma_start(asm_sb[:], asm[:])

    def sc(col):  # f32 per-partition scalar [100,1]
        return scal_sb[0:DCS, col:col + 1]

    # scaled 100x100 identities for the PE-side gate mix:
    # cols j=5k+ri hold m1[k,ri]/r * I, cols 15+k hold m2[k] * I
    mI = consts.tile([DCS, 18, DCS], DT)
    for j in range(18):
        scol = (SC_M1 + j) if j < 15 else (SC_M2 + j - 15)
        nc.vector.tensor_scalar_mul(mI[:, j, :], ident[0:DCS, 0:DCS], sc(scol))

    ans_sb = pers.tile([DCS, BPC, 6, 4], F32, tag="ans_sb")

    # ---------------- gathers + transposes (both b first) --------------
    artT = []
    og = []
    qT = pers.tile([DCS, DC, BPC, TQ], DT, tag="qT")
    oT = pers.tile([DCS, DC, BPC, 4, TO], DT, tag="oT")
    HT = T // 2  # artT halves: zob can start after 8 gathers
    for b in range(BPC):
        at = [p_art.tile([DCS, DC, HT], DT, tag=f"artT{b}_{h}",
                         name=f"artT{b}_{h}") for h in range(2)]
        artT.append(at)
        for c in range(NTCH):
            pc = TCH[c]
            g = gpool.tile([128, DIM], DT, tag="gart", name="gart")
            nc.gpsimd.indirect_dma_start(
                out=g[:pc, :], out_offset=None, in_=emb[:],
                in_offset=bass.IndirectOffsetOnAxis(ap=aidx[b][:pc, c:c + 1],
                                                    axis=0))
            h, hoff = (c * 128) // HT, (c * 128) % HT
            for dc in range(DC):
                tp = ppwork.tile([DCS, 128], DT, tag="work")
                nc.tensor.transpose(tp[:, :pc],
                                    g[:pc, dc * DCS:(dc + 1) * DCS],
                                    ident[:pc, :pc])
                if hoff + pc <= HT:
                    nc.vector.tensor_copy(at[h][:, dc, hoff:hoff + pc],
                                          tp[:, :pc])
                else:
                    n0 = HT - hoff
                    nc.vector.tensor_copy(at[h][:, dc, hoff:HT], tp[:, :n0])
                    nc.vector.tensor_copy(at[h + 1][:, dc, 0:pc - n0],
                                          tp[:, n0:pc])

        qgb = pers.tile([TQ, DIM], DT, tag=f"qg{b}", name=f"qg{b}")
        nc.gpsimd.indirect_dma_start(
            out=qgb[:], out_offset=None, in_=emb[:],
            in_offset=bass.IndirectOffsetOnAxis(ap=qidx[:, b:b + 1], axis=0))
        for dc in range(DC):
            tp = ppwork.tile([DCS, 128], DT, tag="work")
            nc.tensor.transpose(tp[:, :TQ], qgb[:, dc * DCS:(dc + 1) * DCS],
                                ident[:TQ, :TQ])
            nc.vector.tensor_copy(qT[:, dc, b, :], tp[:, :TQ])

        ogb = [pers.tile([TO, DIM], DT, tag=f"og{b}_{o}", name=f"og{b}_{o}")
               for o in range(4)]
        og.append(ogb)
        for o in range(4):
            nc.gpsimd.indirect_dma_start(
                out=ogb[o][:], out_offset=None, in_=emb[:],
                in_offset=bass.IndirectOffsetOnAxis(ap=oidx[:, b, o:o + 1],
                                                    axis=0))
            for dc in range(DC):
                tp = ppwork.tile([DCS, 128], DT, tag="work")
                nc.tensor.transpose(tp[:, :TO],
                                    ogb[o][:, dc * DCS:(dc + 1) * DCS],
                                    ident[:TO, :TO])
                nc.vector.tensor_copy(oT[:, dc, b, o, :], tp[:, :TO])

    def drain_relu(dst, ps, bias_ap, eng):
        """psum -> sbuf relu(x + bias). eng: 's' Scalar ACT, 'v' DVE."""
        if eng == "s":
            nc.scalar.activation(dst, ps, AF.Relu, bias=bias_ap)
        else:
            nc.vector.tensor_scalar(dst, ps, bias_ap, 0.0,
                                    op0=OP.add, op1=OP.max)

    # ---------------- main per-b stream ----------------
    encT = []
    GSZ = (T // 2, T // 4, T // 10, T // 25)
    for b in range(BPC):
        at = artT[b]
        o_full = p_enc.tile([DCS, DC, T], DT, tag=f"enc{b}", name=f"enc{b}")
        encT.append(o_full)

        # ---- phase A: group sums for all 3 d-chunks ----
        xs2 = p_tr.tile([DCS, DC, T // 2], DT, tag="xs2", name="xs2")
        xs4 = p_tr.tile([DCS, DC, T // 4], DT, tag="xs4", name="xs4")
        xs10 = p_tr.tile([DCS, DC, T // 10], DT, tag="xs10", name="xs10")
        xs25 = p_tr.tile([DCS, DC, T // 25], DT, tag="xs25", name="xs25")
        xs = (xs2, xs4, xs10, xs25)
        for dc in range(DC):
            for h in range(2):
                a = at[h][:, dc, :]
                q4 = T // 4
                nc.gpsimd.tensor_add(xs2[:, dc, h * q4:(h + 1) * q4],
                                     a[:, 0:HT:2], a[:, 1:HT:2])
                with nc.allow_low_precision(reason="bf16 group sums"):
                    nc.vector.tensor_reduce(
                        xs25[:, dc, h * (HT // 25):(h + 1) * (HT // 25)],
                        a[:].rearrange("p (g r) -> p g r", r=25),
                        AX.X, OP.add)
            nc.gpsimd.tensor_add(xs4[:, dc, :], xs2[:, dc, 0:T // 2:2],
                                 xs2[:, dc, 1:T // 2:2])
            with nc.allow_low_precision(reason="bf16 group sums"):
                nc.vector.tensor_reduce(
                    xs10[:, dc, :],
                    xs2[:, dc, :].rearrange("p (g r) -> p g r", r=5),
                    AX.X, OP.add)
            if b == 0 and dc == 0:
                dump("d_artT", at[0][:, 0, 0:128])
                dump("d_xs2", xs2[:, 0, 0:64])
                dump("d_xs25", xs25[:, 0, :])

        # ---- phase B: dc-local pipeline ----
        for dc in range(DC):
            # z / o / B1 for this output-dc
            z_sb = p_tr.tile([DCS, T], DT, tag="z", name="z_sb")
            b1_sb = p_tr.tile([DCS, T], DT, tag="b1", name="b1_sb")
            for mi, (dst, func, bcol) in enumerate(
                    ((z_sb[:], AF.Tanh, 0), (o_full[:, dc, :], AF.Tanh, 1),
                     (b1_sb[:], AF.Relu, 2))):
                mcol = WC_ART + mi * DIM + dc * DCS
                for t0 in range(0, T, N_MM):
                    ps = pp500.tile([DCS, N_MM], F32, tag="mm")
                    for kc in range(DC):
                        nc.tensor.matmul(
                            ps[:], w_sb[:, kc, mcol:mcol + DCS],
                            at[t0 // HT][:, kc, t0 % HT:t0 % HT + N_MM],
                            start=(kc == 0), stop=(kc == DC - 1))
                    if mi == 2:
                        drain_relu(dst[:, t0:t0 + N_MM], ps[:],
                                   bias_sb[:, dc, bcol:bcol + 1], "s")
                    else:
                        nc.scalar.activation(
                            dst[:, t0:t0 + N_MM], ps[:], func,
                            bias=bias_sb[:, dc, bcol:bcol + 1])

            # CE r>=2 for this output-dc
            bl = p_tr.tile([DCS, sum(GSZ)], DT, tag="bl", name="bl")
            boff = [0, T // 2, T // 2 + T // 4, T // 2 + T // 4 + T // 10]
            for ri in range(4):
                g_r = GSZ[ri]
                wcol = WC_CE + ri * DIM + dc * DCS
                for j, g0 in enumerate(range(0, g_r, N_MM)):
                    gn = min(N_MM, g_r - g0)
                    ps = pp500.tile([DCS, N_MM], F32, tag="mm")
                    for kc in range(DC):
                        nc.tensor.matmul(
                            ps[:, :gn], w_sb[:, kc, wcol:wcol + DCS],
                            xs[ri][:, kc, g0:g0 + gn],
                            start=(kc == 0), stop=(kc == DC - 1))
                    drain_relu(bl[:, boff[ri] + g0:boff[ri] + g0 + gn],
                               ps[:, :gn], bias_sb[:, dc, 3 + ri:4 + ri],
                               "s" if (ri + j) % 2 == 0 else "v")

            if b == 0 and dc == 0:
                dump("d_z", z_sb[:, 0:128])
                dump("d_b1", b1_sb[:, 0:128])
                dump("d_bl", bl[:])

            def ev_chunk(ri, t0, tn):
                r = RANGES[ri]
                if r == 1:
                    return b1_sb[:, t0:t0 + tn]
                return bl[:, boff[ri - 1] + t0 // r:
                          boff[ri - 1] + (t0 + tn) // r, None] \
                    .to_broadcast([DCS, tn // r, r])

            # gate mix: h1_k = relu(sum_r m1[k,r]/r * B_r^expand + m1_b[k]);
            # gate = relu(sum_k m2[k] h1_k + m2_b). All on PE via scaled
            # identities; expansion = stride-0 rhs views.
            h1 = []
            for k in range(3):
                acc = p_tr.tile([DCS, T], DT, tag=f"h1_{k}", name=f"h1_{k}")
                for t0 in range(0, T, N_MM):
                    ps = pp500.tile([DCS, N_MM], F32, tag="mm")
                    for ri in range(5):
                        nc.tensor.matmul(ps[:], mI[:, 5 * k + ri, :],
                                         ev_chunk(ri, t0, N_MM),
                                         start=(ri == 0), stop=(ri == 4))
                    drain_relu(acc[:, t0:t0 + N_MM], ps[:], sc(SC_M1B + k),
                               "s" if k == 0 or (k == 2 and t0 < T // 2)
                               else "v")
                h1.append(acc)
            if b == 0 and dc == 0:
                dump("d_h10", h1[0][:, 0:128])
                dump("d_h11", h1[1][:, 0:128])
                dump("d_h12", h1[2][:, 0:128])
            gate = p_tr.tile([DCS, T], DT, tag="gate", name="gate")
            for t0 in range(0, T, N_MM):
                ps = pp500.tile([DCS, N_MM], F32, tag="mm")
                for k in range(3):
                    nc.tensor.matmul(ps[:], mI[:, 15 + k, :],
                                     h1[k][:, t0:t0 + N_MM],
                                     start=(k == 0), stop=(k == 2))
                drain_relu(gate[:, t0:t0 + N_MM], ps[:], sc(SC_M2B),
                           "s" if t0 >= T // 2 else "v")

            # MRU scan + encode for this dc
            if b == 0 and dc == 0:
                dump("d_gate", gate[:, 0:128])
            gz = p_tr.tile([DCS, T], DT, tag="gz", name="gz")
            nc.gpsimd.tensor_tensor(gz[:], gate[:], z_sb[:], op=OP.mult)
            nc.vector.tensor_sub(z_sb[:], z_sb[:], gz[:])  # (1-g)z
            c_t = p_tr.tile([DCS, T], DT, tag="c", name="c_t")
            nc.vector.tensor_tensor_scan(
                c_t[:], gate[:], z_sb[:], 0.0, op0=OP.mult, op1=OP.add)
            if b == 0 and dc == 0:
                dump("d_ct", c_t[:, 0:128])
                dump("d_ctend", c_t[:, T - 128:T])
            nc.vector.tensor_mul(o_full[:, dc, :], o_full[:, dc, :], c_t[:])
            if b == 0 and dc == 0:
                dump("d_enc", o_full[:, 0, 0:128])
            if b == 0 and dc == 1:
                dump("d_enc1", o_full[:, 1, 0:128])
            if b == 0 and dc == 2:
                dump("d_enc2", o_full[:, 2, 0:128])

    # ---------------- attention ----------------
    k1T, qk_sb = [], []
    for b in range(BPC):
        # keys1T = f1W @ q^T  (3 output-dc chunks)
        kT = pers.tile([DCS, DC, TQ], DT, tag=f"k1T{b}")
        k1T.append(kT)
        for dc in range(DC):
            ps = ppwork.tile([DCS, TQ], F32, tag="work")
            for kc in range(DC):
                nc.tensor.matmul(ps[:],
                                 w_sb[:, kc, WC_F1 + dc * DCS:
                                      WC_F1 + (dc + 1) * DCS],
                                 qT[:, kc, b, :],
                                 start=(kc == 0), stop=(kc == DC - 1))
            nc.scalar.copy(kT[:, dc, :], ps[:])

        # A2/A3 = q @ fW  then transpose; QK[q, (fi,o,w)] accumulation
        aTs = []
        for fi, wc in enumerate((WC_F2, WC_F3)):
            a_ps = ppwork.tile([TQ, DIM], F32, tag="work")
            for kc in range(DC):
                nc.tensor.matmul(a_ps[:], qT[:, kc, b, :],
                                 w_sb[:, kc, wc:wc + DIM],
                                 start=(kc == 0), stop=(kc == DC - 1))
            a_sb = small.tile([TQ, DIM], DT, tag="a_sb")
            nc.vector.tensor_copy(a_sb[:], a_ps[:])
            aT = small.tile([DCS, DC, TQ], DT, tag=f"aT{fi}")
            for dc in range(DC):
                tp = ppwork.tile([DCS, 128], DT, tag="work")
                nc.tensor.transpose(tp[:, :TQ],
                                    a_sb[:, dc * DCS:(dc + 1) * DCS],
                                    ident[:TQ, :TQ])
                nc.vector.tensor_copy(aT[:, dc, :], tp[:, :TQ])
            aTs.append(aT)

        qk_ps = ppwork.tile([TQ, 128], F32, tag="work")
        for fi in range(2):
            for o in range(4):
                gcol = 16 * (4 * fi + o)
                for kc in range(DC):
                    nc.tensor.matmul(qk_ps[:, gcol:gcol + 16],
                                     aTs[fi][:, kc, :], oT[:, kc, b, o, :],
                                     start=(kc == 0), stop=(kc == DC - 1))
        qk = pers.tile([TQ, 129], DT, tag=f"qk_sb{b}")
        qk_sb.append(qk)
        nc.vector.tensor_copy(qk[:, 0:128], qk_ps[:])
        nc.vector.memset(qk[:, 128:129], 1.0)  # ones col: u2[:,128] = Z1
        if b == 0:
            dump("d_k1T", kT[:].rearrange("p a b -> p (a b)"))
            dump("d_qk", qk[:])

    # attention stream over t chunks; pb[(fi,o)] columns accumulate in PSUM.
    # start=True clears has_written for the WHOLE bank, so interleaved
    # column-groups must instead memset once and accumulate with
    # start=False throughout (accumulate-onto-zero == overwrite).
    pb_ps = [ppacc.tile([TO, 8], F32, tag="pb", name=f"pb{b}")
             for b in range(BPC)]
    for b in range(BPC):
        nc.vector.memset(pb_ps[b][:], 0.0)
    for c in range(NTCH):
        pc = TCH[c]
        for b in range(BPC):
            s1 = pp500.tile([TQ, 128], F32, tag="mm")
            for dc in range(DC):
                nc.tensor.matmul(s1[:, :pc], k1T[b][:, dc, :],
                                 encT[b][:, dc, c * 128:c * 128 + pc],
                                 start=(dc == 0), stop=(dc == DC - 1))
            if b == 0 and c == 0 and dbg:
                dump("d_s1", s1[:])
            e1T = small.tile([TQ, 128], DT, tag="e1Ts")
            nc.scalar.activation(e1T[:, :pc], s1[:, :pc], AF.Exp)
            if b == 0 and c == 0 and dbg:
                dump("d_e1", e1T[:])
            u2 = pp500.tile([128, 129], F32, tag="mm")
            nc.tensor.matmul(u2[:pc, :], e1T[:, :pc], qk_sb[b][:],
                             start=True, stop=True)
            z1 = small.tile([128, 2], F32, tag="z1")
            nc.vector.reciprocal(z1[:pc, 1:2], u2[:pc, 128:129])
            e2 = small.tile([128, 128], DT, tag="e2")
            nc.scalar.activation(e2[:pc, :], u2[:pc, 0:128], AF.Exp,
                                 scale=z1[:pc, 1:2])
            z2 = small.tile([128, 16], DT, tag="z2")
            with nc.allow_low_precision(reason="bf16 attn weights"):
                nc.vector.tensor_reduce(
                    z2[:pc, 0:8],
                    e2[:pc, :].rearrange("p (g w) -> p g w", w=16),
                    AX.X, OP.add)
                nc.vector.reciprocal(z2[:pc, 8:16], z2[:pc, 0:8])
            if b == 0 and c == 0 and dbg:
                dump("d_e2", e2[:])
                dump("d_z2", z2[:])
            for g in range(8):
                nc.tensor.matmul(pb_ps[b][:, g:g + 1],
                                 e2[:pc, 16 * g:16 * g + 16],
                                 z2[:pc, 8 + g:9 + g],
                                 start=False, stop=(c == NTCH - 1),
                                 skip_group_check=True)

    # ---------------- answer vectors ----------------
    for b in range(BPC):
        pb_sb = small.tile([TO, 8], DT, tag="pb_sb")
        nc.vector.tensor_copy(pb_sb[:], pb_ps[b][:])
        if b == 0 and dbg:
            dump("d_pb", pb_ps[b][:])
        ans_ps = ppacc.tile([DCS, 24], F32, tag="pb", name=f"ans{b}")
        for g in range(8):
            fi, o = g // 4, g % 4
            for dc in range(DC):
                j = fi * 3 + dc
                nc.tensor.matmul(ans_ps[:, j * 4 + o:j * 4 + o + 1],
                                 og[b][o][:, dc * DCS:(dc + 1) * DCS],
                                 pb_sb[:, g:g + 1],
                                 start=True, stop=True)
        # 1/T of the mean-over-t lands here
        nc.vector.tensor_scalar_mul(
            ans_sb[:, b, :, :].rearrange("p j o -> p (j o)"), ans_ps[:],
            1.0 / T)

    if dbg:
        dump("d_ans", ans_sb[:].rearrange("p a b c -> p (a b c)"))
    # ---------------- final MLP (both batches together) ----------------
    h_ps = ppwork.tile([75, 8], F32, tag="work")
    for j in range(6):
        # rhs columns = (b, o) pairs for chunk j of the 600-dim ans vector
        nc.tensor.matmul(h_ps[:], w_as1_sb[:, j, :], ans_sb[:, :, j, :],
                         start=(j == 0), stop=(j == 5))
    h_sb = small.tile([75, 8], F32, tag="h_sb")
    nc.scalar.activation(h_sb[:], h_ps[:], AF.Relu, bias=asm_sb[:, 0:1])
    s_ps = ppwork.tile([8, 1], F32, tag="work")
    nc.tensor.matmul(s_ps[:], h_sb[:], asm_sb[:, 1:2], start=True, stop=True)
    s_sb = small.tile([8, 1], F32, tag="s_sb")
    nc.scalar.activation(s_sb[:], s_ps[:], AF.Identity,
                         bias=scal_sb[0:8, SC_AS2B:SC_AS2B + 1])
    nc.sync.dma_start(out[:].rearrange("b o -> (b o)")[:, None], s_sb[:])


# ---------------------------------------------------------------------------
# host side
# ---------------------------------------------------------------------------

_CACHE = {}


def _get_nc():
    if "nc" not in _CACHE:
        _CACHE["nc"] = _build_program()
    return _CACHE["nc"]


def _prep_core_inputs(inputs, core):
    b0 = core * BPC
    sl = slice(b0, b0 + BPC)
    f = np.asarray
    prep = _CACHE.get("prep_shared")
    if prep is None:
        # core-independent tensors, computed once per kernel() call set
        Wz, Wo = f(inputs["Wz"]), f(inputs["Wo"])
        ceW = f(inputs["ce_W"])
        wpack = np.concatenate(
            [Wz.T, Wo.T, ceW[0].T] + [ceW[i].T for i in (1, 2, 3, 4)]
            + [f(inputs["f1_W"]).T, f(inputs["f2_W"]), f(inputs["f3_W"])],
            axis=1)
        fpack = np.stack(
            [f(inputs["bz"]), f(inputs["bo"]),
             *[f(inputs["ce_b"])[i] for i in range(5)]], axis=1)
        prep = {
            "emb": f(inputs["emb"]).astype(NPDT),
            "wpack": np.ascontiguousarray(wpack).astype(NPDT),
            "fpack": np.ascontiguousarray(fpack).astype(np.float32),
            "as1p": np.ascontiguousarray(f(inputs["as1_W"]).T).astype(
                np.float32),
            "asm": np.stack([f(inputs["as1_b"]),
                             f(inputs["as2_W"])[0]], axis=1).astype(
                np.float32),
        }
        scal = np.zeros((128, SC_NCOL), np.float32)
        m1 = f(inputs["mr1_W"])
        for k in range(3):
            for ri, r in enumerate(RANGES):
                scal[:, SC_M1 + 5 * k + ri] = m1[k, ri] / r
        scal[:, SC_M1B:SC_M1B + 3] = f(inputs["mr1_b"])[None, :]
        scal[:, SC_M2:SC_M2 + 3] = f(inputs["mr2_W"])[0][None, :]
        scal[:, SC_M2B] = f(inputs["mr2_b"])[0]
        scal[:, SC_AS2B] = f(inputs["as2_b"])[0]
        prep["scal"] = scal
        _CACHE["prep_shared"] = prep

    d = dict(prep)
    d["art_idx"] = f(inputs["article_in"])[sl].astype(np.int32)
    d["q_idx"] = f(inputs["question_in"])[sl].astype(np.int32)
    d["opt_idx"] = np.stack(
        [f(inputs[f"option{i}_in"])[sl] for i in (1, 2, 3, 4)],
        axis=1).astype(np.int32)
    return d


def _get_runner():
    """jit-compiled 8-core runner, built once per process."""
    if "runner" in _CACHE:
        return _CACHE["runner"]
    import jax
    from jax.sharding import Mesh, PartitionSpec
    from jax.experimental.shard_map import shard_map
    from concourse.bass2jax import (_bass_exec_p, install_neuronx_cc_hook,
                                    partition_id_tensor)

    install_neuronx_cc_hook()
    nc = _get_nc()
    pid_name = nc.partition_id_tensor.name if nc.partition_id_tensor else None

    in_names, out_names, out_avals, zero_outs = [], [], [], []
    for alloc in nc.m.functions[0].allocations:
        if not isinstance(alloc, mybir.MemoryLocationSet):
            continue
        name = alloc.memorylocations[0].name
        if alloc.kind == "ExternalInput":
            if name != pid_name:
                in_names.append(name)
        elif alloc.kind == "ExternalOutput":
            out_names.append(name)
            shape = tuple(alloc.tensor_shape)
            dtype = mybir.dt.np(alloc.dtype)
            out_avals.append(jax.core.ShapedArray(shape, dtype))
            zero_outs.append(np.zeros(shape, dtype))
    n_params = len(in_names)
    all_in_names = in_names + out_names
    if pid_name is not None:
        all_in_names = all_in_names + [pid_name]

    def _body(*args):
        operands = list(args)
        if pid_name is not None:
            operands.append(partition_id_tensor())
        outs = _bass_exec_p.bind(
            *operands, out_avals=tuple(out_avals), in_names=tuple(all_in_names),
            out_names=tuple(out_names), lowering_input_output_aliases=(),
            sim_require_finite=True, sim_require_nnan=True, nc=nc)
        return tuple(outs)

    devices = jax.devices()[:NCORES]
    mesh = Mesh(np.asarray(devices), ("core",))
    in_specs = (PartitionSpec("core"),) * (n_params + len(out_names))
    out_specs = (PartitionSpec("core"),) * len(out_names)
    sharded = jax.jit(shard_map(_body, mesh=mesh, in_specs=in_specs,
                                out_specs=out_specs, check_rep=False),
                      keep_unused=True)

    _CACHE["runner"] = (sharded, in_names, out_names, zero_outs)
    return _CACHE["runner"]


def run_cores(per_core_inputs):
    """per_core_inputs: list of 8 dicts name->np array. Returns out dicts."""
    sharded, in_names, out_names, zero_outs = _get_runner()
    concat_in = [np.concatenate([per_core_inputs[c][n] for c in range(NCORES)],
                                axis=0) for n in in_names]
    concat_zeros = [np.concatenate([z] * NCORES, axis=0) for z in zero_outs]
    outs = sharded(*concat_in, *concat_zeros)
    result = []
    for c in range(NCORES):
        d = {}
        for i, n in enumerate(out_names):
            arr = np.asarray(outs[i])
            per = arr.shape[0] // NCORES
            d[n] = arr[c * per:(c + 1) * per]
        result.append(d)
    return result


def prepare_device_args(per_core_inputs):
    """device_put the concatenated inputs once, for repeated timed runs."""
    import jax
    from jax.sharding import Mesh, PartitionSpec, NamedSharding
    sharded, in_names, out_names, zero_outs = _get_runner()
    devices = jax.devices()[:NCORES]
    mesh = Mesh(np.asarray(devices), ("core",))
    sh = NamedSharding(mesh, PartitionSpec("core"))
    concat_in = [np.concatenate([per_core_inputs[c][n] for c in range(NCORES)],
                                axis=0) for n in in_names]
    concat_zeros = [np.concatenate([z] * NCORES, axis=0) for z in zero_outs]
    args = [jax.device_put(a, sh) for a in concat_in + concat_zeros]
    jax.block_until_ready(args)
    return args


def run_prepared(dev_args):
    sharded, in_names, out_names, zero_outs = _get_runner()
    outs = sharded(*dev_args)
    import jax
    jax.block_until_ready(outs)
    return outs


def kernel(**inputs):
    _CACHE.pop("prep_shared", None)
    per_core = [_prep_core_inputs(inputs, c) for c in range(NCORES)]
    res = run_cores(per_core)
    out = np.concatenate([res[c]["scores"] for c in range(NCORES)], axis=0)
    return out.astype(np.float32)


# revision 22
# speedup vs baseline: 1.3760x; 1.0773x over previous
"""BiAttentionMRU Trainium2 kernel.

Data-parallel over batch: B=16 -> 2 batch elements on each of 8 cores.
All weights replicated. Embedding gather done on-device via indirect DMA.

Layouts (per core, per batch element b in {0,1}):
  art gathered as [t-chunk(128), d=300], PE-transposed into artT[100, 3, 2000]
  (d on partitions, 3 chunks of 100). Group sums, z/o/CE matmuls, gate mix
  (all on PE as scaled-identity accumulating matmuls), MRU scan (native
  tensor_tensor_scan along t) and the attention lhsT all work in [d, t].

Attention algebra: aoq is never materialized. With e1 = exp(art_enc @ keys1^T),
Z1 its row sum, s2 = softmax-normalized p1 @ (q @ keys_f^T) is computed as
exp-of(u2 * 1/Z1) where u2 = e1 @ QK. The per-option mean over t of
softmax(s2) @ opt folds into one accumulating matmul sum_t e2[t,:] * (1/Z2[t]),
accumulated per (fi,o) column directly in PSUM (no partition-shift DMAs).

f1/f2/f3 biases are constant over their softmax axis (shift invariance)
and are dropped.

Per-b flow: phase A stashes the four group-sum tensors for all 3 d-chunks;
phase B runs a dc-local pipeline zob -> CE -> gate mix -> MRU scan so all
big transients recycle through bufs=2 pool tags.
"""

import sys

sys.path.insert(0, "/opt/trn_rl_repo")

import numpy as np
import ml_dtypes

import concourse.bass as bass
import concourse.tile as tile
from concourse import bacc, mybir
from concourse.masks import make_identity

F32 = mybir.dt.float32
BF16 = mybir.dt.bfloat16
I32 = mybir.dt.int32
AX = mybir.AxisListType
OP = mybir.AluOpType
AF = mybir.ActivationFunctionType

DIM = 300
VOCAB = 50000
B_FULL = 16
NCORES = 8
BPC = B_FULL // NCORES  # batch per core = 2
T = 2000
TQ = 30
TO = 16
RANGES = (1, 2, 4, 10, 25)

TCH = [128] * 15 + [80]  # t chunking for transposes / attention
NTCH = len(TCH)
DC = 3  # d chunks of 100
DCS = 100

N_MM = 500  # matmul N-chunking for [d,t] streams (psum free <= 512 fp32)

USE_BF16 = True
DT = BF16 if USE_BF16 else F32
NPDT = ml_dtypes.bfloat16 if USE_BF16 else np.float32

# packed weight columns (bf16, [DIM, WCOLS] host-packed)
WC_ART = 0        # 900: Wz.T | Wo.T | ce0.T
WC_CE = 900       # 1200: ce1..ce4 transposed
WC_F1 = 2100      # 300: f1_W.T
WC_F2 = 2400      # 300: f2_W (untransposed)
WC_F3 = 2700      # 300: f3_W (untransposed)
WCOLS = 3000

# packed f32 bias columns ([DIM, FCOLS]): 0 bz, 1 bo, 2..6 ce_b[0..4]
FCOLS = 7

# scalar table columns (host-packed, replicated down 128 partitions)
SC_M1 = 0      # 15 cols: m1[k,r]/r at 5k+ri
SC_M1B = 15    # 3 cols
SC_M2 = 18     # 3 cols
SC_M2B = 21    # 1 col
SC_AS2B = 22   # 1 col
SC_NCOL = 24


DEBUG = False  # adds intermediate DRAM dumps (b=0) for numerics bisection
_DBG_SPECS = [
    ("d_artT", [DCS, 128]), ("d_xs2", [DCS, 64]), ("d_xs25", [DCS, 80]),
    ("d_z", [DCS, 128]), ("d_b1", [DCS, 128]), ("d_bl", [DCS, 1780]),
    ("d_h10", [DCS, 128]), ("d_h11", [DCS, 128]), ("d_h12", [DCS, 128]),
    ("d_gate", [DCS, 128]), ("d_ct", [DCS, 128]), ("d_ctend", [DCS, 128]),
    ("d_enc", [DCS, 128]), ("d_k1T", [DCS, DC * TQ]), ("d_qk", [TQ, 128]),
    ("d_pb", [TO, 8]), ("d_ans", [DCS, BPC * 24]),
    ("d_enc1", [DCS, 128]), ("d_enc2", [DCS, 128]),
    ("d_s1", [TQ, 128]), ("d_e1", [TQ, 128]), ("d_e2", [128, 128]),
    ("d_z2", [128, 16]),
]


def _build_program():
    nc = bacc.Bacc("TRN2", target_bir_lowering=False, debug=False,
                   num_devices=NCORES)

    emb = nc.dram_tensor("emb", [VOCAB, DIM], DT, kind="ExternalInput")
    art_idx = nc.dram_tensor("art_idx", [BPC, T], I32, kind="ExternalInput")
    q_idx = nc.dram_tensor("q_idx", [BPC, TQ], I32, kind="ExternalInput")
    opt_idx = nc.dram_tensor("opt_idx", [BPC, 4, TO], I32, kind="ExternalInput")
    wpack = nc.dram_tensor("wpack", [DIM, WCOLS], DT, kind="ExternalInput")
    fpack = nc.dram_tensor("fpack", [DIM, FCOLS], F32, kind="ExternalInput")
    as1p = nc.dram_tensor("as1p", [2 * DIM, 75], F32, kind="ExternalInput")
    asm = nc.dram_tensor("asm", [75, 2], F32, kind="ExternalInput")
    scal = nc.dram_tensor("scal", [128, SC_NCOL], F32, kind="ExternalInput")
    out = nc.dram_tensor("scores", [BPC, 4], F32, kind="ExternalOutput")
    dbg = {}
    if DEBUG:
        for nm, shp in _DBG_SPECS:
            dbg[nm] = nc.dram_tensor(nm, shp, F32, kind="ExternalOutput")

    with tile.TileContext(nc) as tc:
        from contextlib import ExitStack
        with ExitStack() as ctx:
            _emit(nc, tc, ctx, emb, art_idx, q_idx, opt_idx, wpack, fpack,
                  as1p, asm, scal, out, dbg)

    nc.compile()
    return nc


def _emit(nc, tc, ctx, emb, art_idx, q_idx, opt_idx, wpack, fpack, as1p,
          asm, scal, out, dbg=None):
    def dump(nm, ap):
        if dbg:
            t = pers.tile(dict(_DBG_SPECS)[nm], F32, tag=f"dmp{nm}")
            nc.vector.tensor_copy(t[:], ap)
            nc.sync.dma_start(dbg[nm][:], t[:])
    # ---------------- pools ----------------
    consts = ctx.enter_context(tc.tile_pool(name="consts", bufs=1))
    gpool = ctx.enter_context(tc.tile_pool(name="gather", bufs=3))
    p_art = ctx.enter_context(tc.tile_pool(name="p_art", bufs=1))
    p_enc = ctx.enter_context(tc.tile_pool(name="p_enc", bufs=1))
    p_tr = ctx.enter_context(tc.tile_pool(name="p_tr", bufs=2))
    pers = ctx.enter_context(tc.tile_pool(name="pers", bufs=1))
    small = ctx.enter_context(tc.tile_pool(name="small", bufs=4))
    # PSUM budget (8 banks): mm(3) + work(3) + pb(2). Every psum tag must
    # stay within this set -- each tag costs bufs banks.
    pp500 = ctx.enter_context(tc.tile_pool(name="pp500", bufs=4, space="PSUM"))
    ppwork = ctx.enter_context(tc.tile_pool(name="ppwork", bufs=2, space="PSUM"))
    ppacc = ctx.enter_context(tc.tile_pool(name="ppacc", bufs=2, space="PSUM"))

    # ---------------- index loads (sync queue, first) ----------------
    aidx = []
    for b in range(BPC):
        ai = small.tile([128, NTCH], I32, tag=f"aidx{b}", name=f"aidx{b}")
        nc.sync.dma_start(ai[:, 0:NTCH - 1],
                          art_idx[b, 0:(NTCH - 1) * 128]
                          .rearrange("(c p) -> p c", p=128))
        nc.sync.dma_start(ai[:TCH[-1], NTCH - 1:NTCH],
                          art_idx[b, (NTCH - 1) * 128:T, None])
        aidx.append(ai)
    qidx = small.tile([TQ, BPC], I32, tag="qidx")
    nc.sync.dma_start(qidx[:], q_idx[:].rearrange("b w -> w b"))
    oidx = small.tile([TO, BPC, 4], I32, tag="oidx")
    nc.sync.dma_start(oidx[:], opt_idx[:].rearrange("b o w -> w b o"))

    # ---------------- constants / weights (sync queue, after idx) ------
    ident = consts.tile([128, 128], DT)
    make_identity(nc, ident[:])

    w_sb = consts.tile([DCS, DC, WCOLS], DT)
    nc.sync.dma_start(w_sb[:], wpack[:].rearrange("(c p) x -> p c x", p=DCS))
    bias_sb = consts.tile([DCS, DC, FCOLS], F32)
    nc.sync.dma_start(bias_sb[:], fpack[:].rearrange("(c p) x -> p c x", p=DCS))
    scal_sb = consts.tile([128, SC_NCOL], F32)
    nc.sync.dma_start(scal_sb[:], scal[:])
    w_as1_sb = consts.tile([DCS, 6, 75], F32)
    nc.sync.dma_start(w_as1_sb[:], as1p[:].rearrange("(c p) x -> p c x", p=DCS))
    asm_sb = consts.tile([75, 2], F32)
    nc.sync.dma_start(asm_sb[:], asm[:])

    def sc(col):  # f32 per-partition scalar [100,1]
        return scal_sb[0:DCS, col:col + 1]

    # scaled 100x100 identities for the PE-side gate mix:
    # cols j=5k+ri hold m1[k,ri]/r * I, cols 15+k hold m2[k] * I
    mI = consts.tile([DCS, 18, DCS], DT)
    for j in range(18):
        scol = (SC_M1 + j) if j < 15 else (SC_M2 + j - 15)
        nc.vector.tensor_scalar_mul(mI[:, j, :], ident[0:DCS, 0:DCS], sc(scol))

    ans_sb = pers.tile([DCS, BPC, 6, 4], F32, tag="ans_sb")

    # ---------------- gathers + transposes (both b first) --------------
    artT = []
    og = []
    qT = pers.tile([DCS, DC, BPC, TQ], DT, tag="qT")
    oT = pers.tile([DCS, DC, BPC, 4, TO], DT, tag="oT")
    HT = T // 2  # artT halves: zob can start after 8 gathers
    for b in range(BPC):
        at = [p_art.tile([DCS, DC, HT], DT, tag=f"artT{b}_{h}",
                         name=f"artT{b}_{h}") for h in range(2)]
        artT.append(at)
        for c in range(NTCH):
            pc = TCH[c]
            g = gpool.tile([128, DIM], DT, tag="gart", name="gart")
            nc.gpsimd.indirect_dma_start(
                out=g[:pc, :], out_offset=None, in_=emb[:],
                in_offset=bass.IndirectOffsetOnAxis(ap=aidx[b][:pc, c:c + 1],
                                                    axis=0))
            h, hoff = (c * 128) // HT, (c * 128) % HT
            for dc in range(DC):
                tp = ppwork.tile([DCS, 128], DT, tag="work")
                nc.tensor.transpose(tp[:, :pc],
                                    g[:pc, dc * DCS:(dc + 1) * DCS],
                                    ident[:pc, :pc])
                if hoff + pc <= HT:
                    nc.vector.tensor_copy(at[h][:, dc, hoff:hoff + pc],
                                          tp[:, :pc])
                else:
                    n0 = HT - hoff
                    nc.vector.tensor_copy(at[h][:, dc, hoff:HT], tp[:, :n0])
                    nc.vector.tensor_copy(at[h + 1][:, dc, 0:pc - n0],
                                          tp[:, n0:pc])

        qgb = pers.tile([TQ, DIM], DT, tag=f"qg{b}", name=f"qg{b}")
        nc.gpsimd.indirect_dma_start(
            out=qgb[:], out_offset=None, in_=emb[:],
            in_offset=bass.IndirectOffsetOnAxis(ap=qidx[:, b:b + 1], axis=0))
        for dc in range(DC):
            tp = ppwork.tile([DCS, 128], DT, tag="work")
            nc.tensor.transpose(tp[:, :TQ], qgb[:, dc * DCS:(dc + 1) * DCS],
                                ident[:TQ, :TQ])
            nc.vector.tensor_copy(qT[:, dc, b, :], tp[:, :TQ])

        ogb = [pers.tile([TO, DIM], DT, tag=f"og{b}_{o}", name=f"og{b}_{o}")
               for o in range(4)]
        og.append(ogb)
        for o in range(4):
            nc.gpsimd.indirect_dma_start(
                out=ogb[o][:], out_offset=None, in_=emb[:],
                in_offset=bass.IndirectOffsetOnAxis(ap=oidx[:, b, o:o + 1],
                                                    axis=0))
            for dc in range(DC):
                tp = ppwork.tile([DCS, 128], DT, tag="work")
                nc.tensor.transpose(tp[:, :TO],
                                    ogb[o][:, dc * DCS:(dc + 1) * DCS],
                                    ident[:TO, :TO])
                nc.vector.tensor_copy(oT[:, dc, b, o, :], tp[:, :TO])

    def drain_relu(dst, ps, bias_ap, eng):
        """psum -> sbuf relu(x + bias). eng: 's' Scalar ACT, 'v' DVE."""
        if eng == "s":
            nc.scalar.activation(dst, ps, AF.Relu, bias=bias_ap)
        else:
            nc.vector.tensor_scalar(dst, ps, bias_ap, 0.0,
                                    op0=OP.add, op1=OP.max)

    # ---------------- main per-b stream ----------------
    encT = []
    GSZ = (T // 2, T // 4, T // 10, T // 25)
    for b in range(BPC):
        at = artT[b]
        o_full = p_enc.tile([DCS, DC, T], DT, tag=f"enc{b}", name=f"enc{b}")
        encT.append(o_full)

        # ---- phase A: group sums for all 3 d-chunks ----
        xs2 = p_tr.tile([DCS, DC, T // 2], DT, tag="xs2", name="xs2")
        xs4 = p_tr.tile([DCS, DC, T // 4], DT, tag="xs4", name="xs4")
        xs10 = p_tr.tile([DCS, DC, T // 10], DT, tag="xs10", name="xs10")
        xs25 = p_tr.tile([DCS, DC, T // 25], DT, tag="xs25", name="xs25")
        xs = (xs2, xs4, xs10, xs25)
        for dc in range(DC):
            for h in range(2):
                a = at[h][:, dc, :]
                q4 = T // 4
                nc.gpsimd.tensor_add(xs2[:, dc, h * q4:(h + 1) * q4],
                                     a[:, 0:HT:2], a[:, 1:HT:2])
                with nc.allow_low_precision(reason="bf16 group sums"):
                    nc.vector.tensor_reduce(
                        xs25[:, dc, h * (HT // 25):(h + 1) * (HT // 25)],
                        a[:].rearrange("p (g r) -> p g r", r=25),
                        AX.X, OP.add)
            nc.gpsimd.tensor_add(xs4[:, dc, :], xs2[:, dc, 0:T // 2:2],
                                 xs2[:, dc, 1:T // 2:2])
            with nc.allow_low_precision(reason="bf16 group sums"):
                nc.vector.tensor_reduce(
                    xs10[:, dc, :],
                    xs2[:, dc, :].rearrange("p (g r) -> p g r", r=5),
                    AX.X, OP.add)
            if b == 0 and dc == 0:
                dump("d_artT", at[0][:, 0, 0:128])
                dump("d_xs2", xs2[:, 0, 0:64])
                dump("d_xs25", xs25[:, 0, :])

        # ---- phase B0: z/o/B1 streams, half-0 chunks for ALL outputs
        # first so the in-order PE queue never waits on half-1 gathers ----
        z_full = p_tr.tile([DCS, DC, T], DT, tag="z", name="z_full")
        b1_full = p_tr.tile([DCS, DC, T], DT, tag="b1", name="b1_full")
        for h in range(2):
            for dc in range(DC):
                for mi, (dst, func, bcol) in enumerate(
                        ((z_full[:, dc, :], AF.Tanh, 0),
                         (o_full[:, dc, :], AF.Tanh, 1),
                         (b1_full[:, dc, :], AF.Relu, 2))):
                    mcol = WC_ART + mi * DIM + dc * DCS
                    for t0 in range(h * HT, (h + 1) * HT, N_MM):
                        ps = pp500.tile([DCS, N_MM], F32, tag="mm")
                        for kc in range(DC):
                            nc.tensor.matmul(
                                ps[:], w_sb[:, kc, mcol:mcol + DCS],
                                at[h][:, kc, t0 % HT:t0 % HT + N_MM],
                                start=(kc == 0), stop=(kc == DC - 1))
                        if mi == 2:
                            drain_relu(dst[:, t0:t0 + N_MM], ps[:],
                                       bias_sb[:, dc, bcol:bcol + 1], "v")
                        else:
                            nc.scalar.activation(
                                dst[:, t0:t0 + N_MM], ps[:], func,
                                bias=bias_sb[:, dc, bcol:bcol + 1])

        # ---- phase B1: per-dc CE -> mix -> scan ----
        for dc in range(DC):
            z_sb = z_full[:, dc, :]
            b1_sb = b1_full[:, dc, :]
            # CE r>=2 for this output-dc
            bl = p_tr.tile([DCS, sum(GSZ)], DT, tag="bl", name="bl")
            boff = [0, T // 2, T // 2 + T // 4, T // 2 + T // 4 + T // 10]
            for ri in range(4):
                g_r = GSZ[ri]
                wcol = WC_CE + ri * DIM + dc * DCS
                for j, g0 in enumerate(range(0, g_r, N_MM)):
                    gn = min(N_MM, g_r - g0)
                    ps = pp500.tile([DCS, N_MM], F32, tag="mm")
                    for kc in range(DC):
                        nc.tensor.matmul(
                            ps[:, :gn], w_sb[:, kc, wcol:wcol + DCS],
                            xs[ri][:, kc, g0:g0 + gn],
                            start=(kc == 0), stop=(kc == DC - 1))
                    drain_relu(bl[:, boff[ri] + g0:boff[ri] + g0 + gn],
                               ps[:, :gn], bias_sb[:, dc, 3 + ri:4 + ri],
                               "s" if (ri + j) % 2 == 0 else "v")

            if b == 0 and dc == 0:
                dump("d_z", z_sb[:, 0:128])
                dump("d_b1", b1_sb[:, 0:128])
                dump("d_bl", bl[:])


            def ev_chunk(ri, t0, tn):
                r = RANGES[ri]
                if r == 1:
                    return b1_sb[:, t0:t0 + tn]
                return bl[:, boff[ri - 1] + t0 // r:
                          boff[ri - 1] + (t0 + tn) // r, None] \
                    .to_broadcast([DCS, tn // r, r])

            # gate mix: h1_k = relu(sum_r m1[k,r]/r * B_r^expand + m1_b[k]);
            # gate = relu(sum_k m2[k] h1_k + m2_b). All on PE via scaled
            # identities; expansion = stride-0 rhs views.
            h1 = []
            for k in range(3):
                acc = p_tr.tile([DCS, T], DT, tag=f"h1_{k}", name=f"h1_{k}")
                for t0 in range(0, T, N_MM):
                    ps = pp500.tile([DCS, N_MM], F32, tag="mm")
                    for ri in range(5):
                        nc.tensor.matmul(ps[:], mI[:, 5 * k + ri, :],
                                         ev_chunk(ri, t0, N_MM),
                                         start=(ri == 0), stop=(ri == 4))
                    drain_relu(acc[:, t0:t0 + N_MM], ps[:], sc(SC_M1B + k),
                               "v" if k == 1 else "s")
                h1.append(acc)
            if b == 0 and dc == 0:
                dump("d_h10", h1[0][:, 0:128])
                dump("d_h11", h1[1][:, 0:128])
                dump("d_h12", h1[2][:, 0:128])
            gate = p_tr.tile([DCS, T], DT, tag="gate", name="gate")
            for t0 in range(0, T, N_MM):
                ps = pp500.tile([DCS, N_MM], F32, tag="mm")
                for k in range(3):
                    nc.tensor.matmul(ps[:], mI[:, 15 + k, :],
                                     h1[k][:, t0:t0 + N_MM],
                                     start=(k == 0), stop=(k == 2))
                drain_relu(gate[:, t0:t0 + N_MM], ps[:], sc(SC_M2B), "s")

            # MRU scan + encode for this dc
            if b == 0 and dc == 0:
                dump("d_gate", gate[:, 0:128])
            c_t = p_tr.tile([DCS, T], DT, tag="c", name="c_t")
            nc.gpsimd.tensor_tensor(c_t[:], gate[:], z_sb, op=OP.mult)  # g*z
            nc.vector.tensor_sub(z_sb, z_sb, c_t[:])  # (1-g)z
            nc.vector.tensor_tensor_scan(
                c_t[:], gate[:], z_sb, 0.0, op0=OP.mult, op1=OP.add)
            if b == 0 and dc == 0:
                dump("d_ct", c_t[:, 0:128])
                dump("d_ctend", c_t[:, T - 128:T])
            nc.vector.tensor_mul(o_full[:, dc, :], o_full[:, dc, :], c_t[:])
            if b == 0 and dc == 0:
                dump("d_enc", o_full[:, 0, 0:128])
            if b == 0 and dc == 1:
                dump("d_enc1", o_full[:, 1, 0:128])
            if b == 0 and dc == 2:
                dump("d_enc2", o_full[:, 2, 0:128])

    # ---------------- attention ----------------
    k1T, qk_sb = [], []
    for b in range(BPC):
        # keys1T = f1W @ q^T  (3 output-dc chunks)
        kT = pers.tile([DCS, DC, TQ], DT, tag=f"k1T{b}")
        k1T.append(kT)
        for dc in range(DC):
            ps = ppwork.tile([DCS, TQ], F32, tag="work")
            for kc in range(DC):
                nc.tensor.matmul(ps[:],
                                 w_sb[:, kc, WC_F1 + dc * DCS:
                                      WC_F1 + (dc + 1) * DCS],
                                 qT[:, kc, b, :],
                                 start=(kc == 0), stop=(kc == DC - 1))
            nc.scalar.copy(kT[:, dc, :], ps[:])

        # A2/A3 = q @ fW  then transpose; QK[q, (fi,o,w)] accumulation
        aTs = []
        for fi, wc in enumerate((WC_F2, WC_F3)):
            a_ps = ppwork.tile([TQ, DIM], F32, tag="work")
            for kc in range(DC):
                nc.tensor.matmul(a_ps[:], qT[:, kc, b, :],
                                 w_sb[:, kc, wc:wc + DIM],
                                 start=(kc == 0), stop=(kc == DC - 1))
            a_sb = small.tile([TQ, DIM], DT, tag="a_sb")
            nc.vector.tensor_copy(a_sb[:], a_ps[:])
            aT = small.tile([DCS, DC, TQ], DT, tag=f"aT{fi}")
            for dc in range(DC):
                tp = ppwork.tile([DCS, 128], DT, tag="work")
                nc.tensor.transpose(tp[:, :TQ],
                                    a_sb[:, dc * DCS:(dc + 1) * DCS],
                                    ident[:TQ, :TQ])
                nc.vector.tensor_copy(aT[:, dc, :], tp[:, :TQ])
            aTs.append(aT)

        qk_ps = ppwork.tile([TQ, 128], F32, tag="work")
        for fi in range(2):
            for o in range(4):
                gcol = 16 * (4 * fi + o)
                for kc in range(DC):
                    nc.tensor.matmul(qk_ps[:, gcol:gcol + 16],
                                     aTs[fi][:, kc, :], oT[:, kc, b, o, :],
                                     start=(kc == 0), stop=(kc == DC - 1))
        qk = pers.tile([TQ, 129], DT, tag=f"qk_sb{b}")
        qk_sb.append(qk)
        nc.vector.tensor_copy(qk[:, 0:128], qk_ps[:])
        nc.vector.memset(qk[:, 128:129], 1.0)  # ones col: u2[:,128] = Z1
        if b == 0:
            dump("d_k1T", kT[:].rearrange("p a b -> p (a b)"))
            dump("d_qk", qk[:])

    # attention stream over t chunks; pb[(fi,o)] columns accumulate in PSUM.
    # start=True clears has_written for the WHOLE bank, so interleaved
    # column-groups must instead memset once and accumulate with
    # start=False throughout (accumulate-onto-zero == overwrite).
    pb_ps = [ppacc.tile([TO, 8], F32, tag="pb", name=f"pb{b}")
             for b in range(BPC)]
    for b in range(BPC):
        nc.vector.memset(pb_ps[b][:], 0.0)
    for c in range(NTCH):
        pc = TCH[c]
        for b in range(BPC):
            s1 = pp500.tile([TQ, 128], F32, tag="mm")
            for dc in range(DC):
                nc.tensor.matmul(s1[:, :pc], k1T[b][:, dc, :],
                                 encT[b][:, dc, c * 128:c * 128 + pc],
                                 start=(dc == 0), stop=(dc == DC - 1))
            if b == 0 and c == 0 and dbg:
                dump("d_s1", s1[:])
            e1T = small.tile([TQ, 128], DT, tag="e1Ts")
            nc.scalar.activation(e1T[:, :pc], s1[:, :pc], AF.Exp)
            if b == 0 and c == 0 and dbg:
                dump("d_e1", e1T[:])
            u2 = pp500.tile([128, 129], F32, tag="mm")
            nc.tensor.matmul(u2[:pc, :], e1T[:, :pc], qk_sb[b][:],
                             start=True, stop=True)
            z1 = small.tile([128, 2], F32, tag="z1")
            nc.vector.reciprocal(z1[:pc, 1:2], u2[:pc, 128:129])
            e2 = small.tile([128, 128], DT, tag="e2")
            nc.scalar.activation(e2[:pc, :], u2[:pc, 0:128], AF.Exp,
                                 scale=z1[:pc, 1:2])
            z2 = small.tile([128, 16], DT, tag="z2")
            with nc.allow_low_precision(reason="bf16 attn weights"):
                nc.vector.tensor_reduce(
                    z2[:pc, 0:8],
                    e2[:pc, :].rearrange("p (g w) -> p g w", w=16),
                    AX.X, OP.add)
                nc.vector.reciprocal(z2[:pc, 8:16], z2[:pc, 0:8])
            if b == 0 and c == 0 and dbg:
                dump("d_e2", e2[:])
                dump("d_z2", z2[:])
            for g in range(8):
                nc.tensor.matmul(pb_ps[b][:, g:g + 1],
                                 e2[:pc, 16 * g:16 * g + 16],
                                 z2[:pc, 8 + g:9 + g],
                                 start=False, stop=(c == NTCH - 1),
                                 skip_group_check=True)

    # ---------------- answer vectors ----------------
    for b in range(BPC):
        pb_sb = small.tile([TO, 8], DT, tag="pb_sb")
        nc.vector.tensor_copy(pb_sb[:], pb_ps[b][:])
        if b == 0 and dbg:
            dump("d_pb", pb_ps[b][:])
        ans_ps = ppacc.tile([DCS, 24], F32, tag="pb", name=f"ans{b}")
        for g in range(8):
            fi, o = g // 4, g % 4
            for dc in range(DC):
                j = fi * 3 + dc
                nc.tensor.matmul(ans_ps[:, j * 4 + o:j * 4 + o + 1],
                                 og[b][o][:, dc * DCS:(dc + 1) * DCS],
                                 pb_sb[:, g:g + 1],
                                 start=True, stop=True)
        # 1/T of the mean-over-t lands here
        nc.vector.tensor_scalar_mul(
            ans_sb[:, b, :, :].rearrange("p j o -> p (j o)"), ans_ps[:],
            1.0 / T)

    if dbg:
        dump("d_ans", ans_sb[:].rearrange("p a b c -> p (a b c)"))
    # ---------------- final MLP (both batches together) ----------------
    h_ps = ppwork.tile([75, 8], F32, tag="work")
    for j in range(6):
        # rhs columns = (b, o) pairs for chunk j of the 600-dim ans vector
        nc.tensor.matmul(h_ps[:], w_as1_sb[:, j, :], ans_sb[:, :, j, :],
                         start=(j == 0), stop=(j == 5))
    h_sb = small.tile([75, 8], F32, tag="h_sb")
    nc.scalar.activation(h_sb[:], h_ps[:], AF.Relu, bias=asm_sb[:, 0:1])
    s_ps = ppwork.tile([8, 1], F32, tag="work")
    nc.tensor.matmul(s_ps[:], h_sb[:], asm_sb[:, 1:2], start=True, stop=True)
    s_sb = small.tile([8, 1], F32, tag="s_sb")
    nc.scalar.activation(s_sb[:], s_ps[:], AF.Identity,
                         bias=scal_sb[0:8, SC_AS2B:SC_AS2B + 1])
    nc.sync.dma_start(out[:].rearrange("b o -> (b o)")[:, None], s_sb[:])


# ---------------------------------------------------------------------------
# host side
# ---------------------------------------------------------------------------

_CACHE = {}


def _get_nc():
    if "nc" not in _CACHE:
        _CACHE["nc"] = _build_program()
    return _CACHE["nc"]


def _prep_core_inputs(inputs, core):
    b0 = core * BPC
    sl = slice(b0, b0 + BPC)
    f = np.asarray
    prep = _CACHE.get("prep_shared")
    if prep is None:
        # core-independent tensors, computed once per kernel() call set
        Wz, Wo = f(inputs["Wz"]), f(inputs["Wo"])
        ceW = f(inputs["ce_W"])
        wpack = np.concatenate(
            [Wz.T, Wo.T, ceW[0].T] + [ceW[i].T for i in (1, 2, 3, 4)]
            + [f(inputs["f1_W"]).T, f(inputs["f2_W"]), f(inputs["f3_W"])],
            axis=1)
        fpack = np.stack(
            [f(inputs["bz"]), f(inputs["bo"]),
             *[f(inputs["ce_b"])[i] for i in range(5)]], axis=1)
        prep = {
            "emb": f(inputs["emb"]).astype(NPDT),
            "wpack": np.ascontiguousarray(wpack).astype(NPDT),
            "fpack": np.ascontiguousarray(fpack).astype(np.float32),
            "as1p": np.ascontiguousarray(f(inputs["as1_W"]).T).astype(
                np.float32),
            "asm": np.stack([f(inputs["as1_b"]),
                             f(inputs["as2_W"])[0]], axis=1).astype(
                np.float32),
        }
        scal = np.zeros((128, SC_NCOL), np.float32)
        m1 = f(inputs["mr1_W"])
        for k in range(3):
            for ri, r in enumerate(RANGES):
                scal[:, SC_M1 + 5 * k + ri] = m1[k, ri] / r
        scal[:, SC_M1B:SC_M1B + 3] = f(inputs["mr1_b"])[None, :]
        scal[:, SC_M2:SC_M2 + 3] = f(inputs["mr2_W"])[0][None, :]
        scal[:, SC_M2B] = f(inputs["mr2_b"])[0]
        scal[:, SC_AS2B] = f(inputs["as2_b"])[0]
        prep["scal"] = scal
        _CACHE["prep_shared"] = prep

    d = dict(prep)
    d["art_idx"] = f(inputs["article_in"])[sl].astype(np.int32)
    d["q_idx"] = f(inputs["question_in"])[sl].astype(np.int32)
    d["opt_idx"] = np.stack(
        [f(inputs[f"option{i}_in"])[sl] for i in (1, 2, 3, 4)],
        axis=1).astype(np.int32)
    return d


def _get_runner():
    """jit-compiled 8-core runner, built once per process."""
    if "runner" in _CACHE:
        return _CACHE["runner"]
    import jax
    from jax.sharding import Mesh, PartitionSpec
    from jax.experimental.shard_map import shard_map
    from concourse.bass2jax import (_bass_exec_p, install_neuronx_cc_hook,
                                    partition_id_tensor)

    install_neuronx_cc_hook()
    nc = _get_nc()
    pid_name = nc.partition_id_tensor.name if nc.partition_id_tensor else None

    in_names, out_names, out_avals, zero_outs = [], [], [], []
    for alloc in nc.m.functions[0].allocations:
        if not isinstance(alloc, mybir.MemoryLocationSet):
            continue
        name = alloc.memorylocations[0].name
        if alloc.kind == "ExternalInput":
            if name != pid_name:
                in_names.append(name)
        elif alloc.kind == "ExternalOutput":
            out_names.append(name)
            shape = tuple(alloc.tensor_shape)
            dtype = mybir.dt.np(alloc.dtype)
            out_avals.append(jax.core.ShapedArray(shape, dtype))
            zero_outs.append(np.zeros(shape, dtype))
    n_params = len(in_names)
    all_in_names = in_names + out_names
    if pid_name is not None:
        all_in_names = all_in_names + [pid_name]

    def _body(*args):
        operands = list(args)
        if pid_name is not None:
            operands.append(partition_id_tensor())
        outs = _bass_exec_p.bind(
            *operands, out_avals=tuple(out_avals), in_names=tuple(all_in_names),
            out_names=tuple(out_names), lowering_input_output_aliases=(),
            sim_require_finite=True, sim_require_nnan=True, nc=nc)
        return tuple(outs)

    devices = jax.devices()[:NCORES]
    mesh = Mesh(np.asarray(devices), ("core",))
    in_specs = (PartitionSpec("core"),) * (n_params + len(out_names))
    out_specs = (PartitionSpec("core"),) * len(out_names)
    sharded = jax.jit(shard_map(_body, mesh=mesh, in_specs=in_specs,
                                out_specs=out_specs, check_rep=False),
                      keep_unused=True)

    _CACHE["runner"] = (sharded, in_names, out_names, zero_outs)
    return _CACHE["runner"]


def run_cores(per_core_inputs):
    """per_core_inputs: list of 8 dicts name->np array. Returns out dicts."""
    sharded, in_names, out_names, zero_outs = _get_runner()
    concat_in = [np.concatenate([per_core_inputs[c][n] for c in range(NCORES)],
                                axis=0) for n in in_names]
    concat_zeros = [np.concatenate([z] * NCORES, axis=0) for z in zero_outs]
    outs = sharded(*concat_in, *concat_zeros)
    result = []
    for c in range(NCORES):
        d = {}
        for i, n in enumerate(out_names):
            arr = np.asarray(outs[i])
            per = arr.shape[0] // NCORES
            d[n] = arr[c * per:(c + 1) * per]
        result.append(d)
    return result


def prepare_device_args(per_core_inputs):
    """device_put the concatenated inputs once, for repeated timed runs."""
    import jax
    from jax.sharding import Mesh, PartitionSpec, NamedSharding
    sharded, in_names, out_names, zero_outs = _get_runner()
    devices = jax.devices()[:NCORES]
    mesh = Mesh(np.asarray(devices), ("core",))
    sh = NamedSharding(mesh, PartitionSpec("core"))
    concat_in = [np.concatenate([per_core_inputs[c][n] for c in range(NCORES)],
                                axis=0) for n in in_names]
    concat_zeros = [np.concatenate([z] * NCORES, axis=0) for z in zero_outs]
    args = [jax.device_put(a, sh) for a in concat_in + concat_zeros]
    jax.block_until_ready(args)
    return args


def run_prepared(dev_args):
    sharded, in_names, out_names, zero_outs = _get_runner()
    outs = sharded(*dev_args)
    import jax
    jax.block_until_ready(outs)
    return outs


def kernel(**inputs):
    _CACHE.pop("prep_shared", None)
    per_core = [_prep_core_inputs(inputs, c) for c in range(NCORES)]
    res = run_cores(per_core)
    out = np.concatenate([res[c]["scores"] for c in range(NCORES)], axis=0)
    return out.astype(np.float32)
